# revision 1
# baseline (speedup 1.0000x reference)
"""Trainium2 Bass kernel for nn_Attention_79645873537262.

Dense attention with per-head bias, key masking, sigmoid gate:
  t = x @ w_proj.T; per head: q,k,v
  a = softmax(scale*q@k.T + bias + mask); y = a@v
  y = sigmoid(x@w_g.T + b_g) * y;  out = y @ w_o.T + b_o

Sharding: tensor-parallel over heads, 2 heads per core on 8 cores.
Each core runs a fully independent program (no collectives): it computes
its 2 heads' attention plus its 128-column slice of the gate, and a
partial o_proj (contribution of its 128 y-columns to all 1024 outputs).
The host sums the 8 partial outputs and adds b_o (the "all-reduce").

On-device layout is transposed ("scores.T" flash style):
  scores.T[k,q] accumulated in PSUM as  ident@biasT (bias pre-masked,
  pre-transposed on host) + kT.T@qT ; exp on ScalarE (no max-subtraction:
  logits are ~N(0,2) by construction, |logit| < ~14 so exp is safe);
  y.T ext = [v | ones].T @ p gives y.T rows 0..63 and the softmax
  denominator in row 64. Normalization multiplies by a broadcast
  reciprocal (DMA DRAM round-trip to cross partitions).
All matmuls run in float32r (full-rate fp32, ~1.5e-4 relative rounding).

Perf notes (from NTFF traces): every f32r matmul pays a serialized
~LDWEIGHTS+MATMUL pair (~426 ns warm); HAM re-warm never triggers in
this instruction mix, so the kernel must never let the PE idle >3.4us:
one PSUM pool layout for all phases (no pool-transition barrier),
proj-critical DMAs dispatched first, bias stream on the (otherwise
idle) GpSimd DGE queue, and a per-q-tile tail so o_proj/output DMA
overlap the end of attention.
"""
import sys
import numpy as np
import ml_dtypes

try:
    import concourse.bass as bass
except ImportError:
    sys.path.insert(0, "/opt/trn_rl_repo")
    import concourse.bass as bass

import concourse.tile as tile
from concourse import bacc, mybir
from concourse.bass_utils import run_bass_kernel_spmd

B, L, E, H = 1, 2048, 1024, 16
HW = E // H                # 64
SCALE = HW ** -0.5
N_CORES = 8
HPC = H // N_CORES         # 2 heads per core
C2 = HPC * HW              # 128 y-columns per core
MASK_NEG = -60.0           # exp(-60 + max_bias) ~ 1e-23: dead keys vanish

f32 = mybir.dt.float32
f32r = mybir.dt.float32r
bf16 = mybir.dt.float16

NE = E // 128              # 8 contraction chunks
NQ = L // 512              # 4 q-tiles of 512
NKT = L // 128             # 16 k-chunks of 128

_compiled = [None]
DEBUG = False


def _build():
    nc = bacc.Bacc("TRN2", target_bir_lowering=False, debug=False,
                   num_devices=N_CORES)

    xT_ap = nc.dram_tensor("xT", [E, L], f32r, kind="ExternalInput").ap()
    wpT_ap = nc.dram_tensor("wpT", [E, 3 * C2], f32r, kind="ExternalInput").ap()
    biasT_ap = nc.dram_tensor("biasT", [HPC, L, L], bf16, kind="ExternalInput").ap()
    wgT_ap = nc.dram_tensor("wgT", [E, C2], f32r, kind="ExternalInput").ap()
    bgv_ap = nc.dram_tensor("bgv", [C2, 1], f32, kind="ExternalInput").ap()
    woT_ap = nc.dram_tensor("woT", [C2, E], f32r, kind="ExternalInput").ap()
    ident_ap = nc.dram_tensor("ident", [128, 128], bf16, kind="ExternalInput").ap()
    onescols_ap = nc.dram_tensor("onescols", [128, NKT], f32r, kind="ExternalInput").ap()
    identr_ap = nc.dram_tensor("identr", [128, 128], f32r, kind="ExternalInput").ap()
    outT_ap = nc.dram_tensor("outT", [E, L], f32, kind="ExternalOutput").ap()

    with tile.TileContext(nc) as tc:
        from contextlib import ExitStack
        with ExitStack() as ctx:
            pers = ctx.enter_context(tc.tile_pool(name="pers", bufs=1))
            work = ctx.enter_context(tc.tile_pool(name="work", bufs=1))
            biasp = ctx.enter_context(tc.tile_pool(name="bias", bufs=4))
            pp = ctx.enter_context(tc.tile_pool(name="pp", bufs=3))
            nrm = ctx.enter_context(tc.tile_pool(name="nrm", bufs=1))
            dramp = ctx.enter_context(tc.tile_pool(name="dram", bufs=4, space="DRAM"))
            outp = ctx.enter_context(tc.tile_pool(name="outp", bufs=3))
            # one PSUM layout for the whole kernel: no pool-transition barrier
            sp = ctx.enter_context(tc.tile_pool(name="s", bufs=2, space="PSUM"))
            yp = ctx.enter_context(tc.tile_pool(name="y", bufs=1, space="PSUM"))

            # --- proj-critical DMAs first (dispatch order matters) ---
            # x and w_proj arrive in per-chunk contiguous pieces so the proj
            # matmuls can start as soon as the first chunks land.
            xT_sb = [pers.tile([128, L], f32r, name=f"xT{e}", tag=f"xT{e}")
                     for e in range(NE)]
            wpT_sb = [pers.tile([128, 3 * C2], f32r, name=f"wpT{e}", tag=f"wpT{e}")
                      for e in range(NE)]
            for e in range(NE):
                nc.sync.dma_start(wpT_sb[e], wpT_ap[e * 128:(e + 1) * 128, :])
                nc.sync.dma_start(xT_sb[e][:, 0:1024],
                                  xT_ap[e * 128:(e + 1) * 128, 0:1024])
            for e in range(NE):
                nc.sync.dma_start(xT_sb[e][:, 1024:2048],
                                  xT_ap[e * 128:(e + 1) * 128, 1024:2048])
            ident_sb = pers.tile([128, 128], bf16, tag="ident")
            nc.sync.dma_start(ident_sb, ident_ap)
            wgT_sb = [pers.tile([128, C2], f32r, name=f"wgT{e}", tag=f"wgT{e}")
                      for e in range(NE)]
            for e in range(NE):
                nc.sync.dma_start(wgT_sb[e], wgT_ap[e * 128:(e + 1) * 128, :])
            bgv_sb = pers.tile([C2, 1], f32, tag="bgv")
            nc.sync.dma_start(bgv_sb, bgv_ap)
            woT_sb = pers.tile([C2, E], f32r, tag="woT")
            nc.sync.dma_start(woT_sb, woT_ap)
            identr_sb = pers.tile([128, 128], f32r, tag="identr")
            nc.sync.dma_start(identr_sb, identr_ap)
            # v tiles: [128 l, 130] per k-chunk: [v_h0 | ones | v_h1 | ones]
            v_all = pers.tile([128, NKT, 130], f32r, tag="v_all")
            nc.sync.dma_start(v_all[:, :, 64:65], onescols_ap.unsqueeze(2))
            nc.sync.dma_start(v_all[:, :, 129:130], onescols_ap.unsqueeze(2))

            q01 = pers.tile([128, L], f32r, tag="q01")
            k01 = pers.tile([128, L], f32r, tag="k01")
            g_sb = pers.tile([128, L], f32r, tag="g")
            ygT = pers.tile([128, L], f32r, tag="ygT")

            # ---------------- proj ----------------
            # e is the weight-change axis; the two inner 512-slices reuse the
            # loaded weight chunk (consecutive same-weight matmuls pipeline at
            # ~227 ns vs ~425 ns when weights change).
            vT01 = work.tile([128, L], f32r, tag="vT01")
            dests = [q01, k01, vT01]
            for lh in range(2):
                for f in range(3):
                    ps = sp.tile([128, 1024], f32, name=f"pj{f}_{lh}", tag="s")
                    for e in range(NE):
                        for ltq in range(2):
                            nc.tensor.matmul(
                                ps[:, ltq * 512:(ltq + 1) * 512],
                                wpT_sb[e][:, f * 128:(f + 1) * 128],
                                xT_sb[e][:, lh * 1024 + ltq * 512:
                                          lh * 1024 + (ltq + 1) * 512],
                                start=(e == 0), stop=(e == NE - 1))
                    nc.vector.tensor_copy(
                        dests[f][:, lh * 1024:(lh + 1) * 1024], ps)

            # transpose vT01 -> v_all[:, kt, :]
            for kt in range(NKT):
                ps = sp.tile([128, 128], f32r, name=f"tr{kt}", tag="s")
                nc.tensor.transpose(
                    ps, vT01[:, kt * 128:(kt + 1) * 128], identr_sb)
                nc.vector.tensor_copy(v_all[:, kt, 0:64], ps[:, 0:64])
                nc.vector.tensor_copy(v_all[:, kt, 65:129], ps[:, 64:128])

            # gate: g = sigmoid(wgT.T @ xT + bg)
            for lh in range(2):
                ps = sp.tile([C2, 1024], f32, name=f"pg{lh}", tag="s")
                for e in range(NE):
                    for ltq in range(2):
                        nc.tensor.matmul(
                            ps[:, ltq * 512:(ltq + 1) * 512], wgT_sb[e],
                            xT_sb[e][:, lh * 1024 + ltq * 512:
                                      lh * 1024 + (ltq + 1) * 512],
                            start=(e == 0), stop=(e == NE - 1))
                nc.scalar.activation(
                    g_sb[:, lh * 1024:(lh + 1) * 1024], ps,
                    mybir.ActivationFunctionType.Sigmoid,
                    bias=bgv_sb, scale=1.0)

            # ---------------- attention: 4 passes over (q-half, head) ----------------
            # y psum double-buffered across passes so pass p+1 accumulates
            # while pass p drains through its normalization chain. The
            # q-half tail (gate mul + o_proj) is emitted one pass late so the
            # PE stream never blocks on the normalization DMA round-trip.
            def attention_pass(qhalf, h):
                hb = h * 64
                y_ps = [yp.tile([65, 512], f32, name=f"y{qhalf}_{h}_{i}",
                                tag=f"y{i}", bufs=2) for i in range(2)]
                for kt in range(NKT):
                    bias_t = biasp.tile([128, 1024], bf16,
                                        name=f"bias{qhalf}_{h}_{kt}", tag="bias")
                    dma_eng = nc.gpsimd if kt % 2 == 0 else nc.sync
                    dma_eng.dma_start(
                        bias_t, biasT_ap[h, kt * 128:(kt + 1) * 128,
                                         qhalf * 1024:(qhalf + 1) * 1024])
                    s_ps = sp.tile([128, 1024], f32,
                                   name=f"s{qhalf}_{h}_{kt}", tag="s")
                    for qq in range(2):
                        nc.tensor.matmul(
                            s_ps[:, qq * 512:(qq + 1) * 512],
                            ident_sb, bias_t[:, qq * 512:(qq + 1) * 512],
                            start=True, stop=False)
                    for qq in range(2):
                        qs = qhalf * 1024 + qq * 512
                        nc.tensor.matmul(
                            s_ps[:, qq * 512:(qq + 1) * 512],
                            k01[hb:hb + 64, kt * 128:(kt + 1) * 128],
                            q01[hb:hb + 64, qs:qs + 512],
                            start=False, stop=True)
                    p_t = pp.tile([128, 1024], f32r,
                                  name=f"p{qhalf}_{h}_{kt}", tag="p")
                    nc.scalar.activation(
                        p_t, s_ps, mybir.ActivationFunctionType.Exp)
                    for qq in range(2):
                        nc.tensor.matmul(
                            y_ps[qq],
                            v_all[:, kt, h * 65:(h + 1) * 65],
                            p_t[:, qq * 512:(qq + 1) * 512],
                            start=(kt == 0), stop=(kt == NKT - 1))
                # normalization chains (softmax denominators in row 64)
                for qq in range(2):
                    qt = qhalf * 2 + qq
                    qsl = slice(qt * 512, (qt + 1) * 512)
                    sums_sb = nrm.tile([65, 512], f32,
                                       name=f"sums{qhalf}_{h}_{qq}", tag="sums")
                    nc.vector.tensor_copy(sums_sb[64:65, :], y_ps[qq][64:65, :])
                    dscr = dramp.tile([1, 512], f32,
                                      name=f"dscr{qhalf}_{h}_{qq}", tag="dscr")
                    nc.sync.dma_start(dscr, sums_sb[64:65, :])
                    sums_b = nrm.tile([64, 512], f32,
                                      name=f"sums_b{qhalf}_{h}_{qq}", tag="sums_b")
                    nc.sync.dma_start(sums_b, dscr.partition_broadcast(64))
                    rb_sb = nrm.tile([64, 512], f32, name=f"rb{qhalf}_{h}_{qq}", tag="rb")
                    nc.vector.reciprocal_approx_fast(rb_sb, sums_b)
                    if h == 0:
                        nc.vector.tensor_mul(
                            ygT[0:64, qsl], y_ps[qq][0:64, :], rb_sb)
                    else:
                        yg1 = nrm.tile([64, 512], f32r,
                                       name=f"yg1_{qhalf}_{qq}", tag="yg1")
                        nc.vector.tensor_mul(yg1, y_ps[qq][0:64, :], rb_sb)
                        nc.sync.dma_start(ygT[64:128, qsl], yg1)

            def qhalf_tail(qhalf):
                # gate multiply + o_proj partial for this q-half
                for qq in range(2):
                    qt = qhalf * 2 + qq
                    qsl = slice(qt * 512, (qt + 1) * 512)
                    nc.vector.tensor_mul(ygT[:, qsl], ygT[:, qsl], g_sb[:, qsl])
                for eo in range(NE):
                    ps = sp.tile([128, 1024], f32, name=f"po{qhalf}_{eo}", tag="s")
                    for qq in range(2):
                        qt = qhalf * 2 + qq
                        nc.tensor.matmul(
                            ps[:, qq * 512:(qq + 1) * 512],
                            woT_sb[:, eo * 128:(eo + 1) * 128],
                            ygT[:, qt * 512:(qt + 1) * 512],
                            start=True, stop=True)
                    ot = outp.tile([128, 1024], f32, name=f"ot{qhalf}_{eo}", tag="ot")
                    if eo % 2 == 0:
                        nc.vector.tensor_copy(ot, ps)
                    else:
                        nc.scalar.copy(ot, ps)
                    nc.sync.dma_start(
                        outT_ap[eo * 128:(eo + 1) * 128,
                                qhalf * 1024:(qhalf + 1) * 1024], ot)

            attention_pass(0, 0)
            attention_pass(0, 1)
            attention_pass(1, 0)
            qhalf_tail(0)
            attention_pass(1, 1)
            qhalf_tail(1)

    nc.compile()
    return nc


def kernel(x, mask, bias, w_proj, w_o, b_o, w_g, b_g):
    x = np.asarray(x, dtype=np.float32)
    mask = np.asarray(mask)
    bias = np.asarray(bias, dtype=np.float32)
    w_proj = np.asarray(w_proj, dtype=np.float32)
    w_o = np.asarray(w_o, dtype=np.float32)
    b_o = np.asarray(b_o, dtype=np.float32)
    w_g = np.asarray(w_g, dtype=np.float32)
    b_g = np.asarray(b_g, dtype=np.float32)

    if _compiled[0] is None:
        _compiled[0] = _build()
    nc = _compiled[0]

    xT = np.ascontiguousarray(x[0].T)                      # [E, L]
    mask_add = np.where(mask[0], 0.0, MASK_NEG).astype(np.float32)  # [L]
    ident = np.eye(128, dtype=np.float16)
    identr = np.eye(128, dtype=np.float32)
    onescols = np.ones((128, NKT), dtype=np.float32)

    in_maps = []
    for c in range(N_CORES):
        heads = [c * HPC + i for i in range(HPC)]
        wpT = np.empty((E, 3 * C2), dtype=np.float32)
        for i, h in enumerate(heads):
            r0 = h * 3 * HW
            wpT[:, 0 * C2 + i * HW: 0 * C2 + (i + 1) * HW] = \
                w_proj[r0: r0 + HW].T * SCALE               # q, pre-scaled
            wpT[:, 1 * C2 + i * HW: 1 * C2 + (i + 1) * HW] = \
                w_proj[r0 + HW: r0 + 2 * HW].T              # k
            wpT[:, 2 * C2 + i * HW: 2 * C2 + (i + 1) * HW] = \
                w_proj[r0 + 2 * HW: r0 + 3 * HW].T          # v
        biasT = np.ascontiguousarray(
            bias[0, :, :, heads].transpose(0, 2, 1))        # [2, Lk, Lq]
        biasT += mask_add[None, :, None]
        biasT = biasT.astype(np.float16)
        cols = slice(c * C2, (c + 1) * C2)
        wgT = np.ascontiguousarray(w_g[cols, :].T)          # [E, C2]
        bgv = np.ascontiguousarray(b_g[cols, None])         # [C2, 1]
        woT = np.ascontiguousarray(w_o[:, cols].T)          # [C2, E]
        in_maps.append({
            "xT": xT, "wpT": wpT, "biasT": biasT, "wgT": wgT,
            "bgv": bgv, "woT": woT, "ident": ident, "identr": identr, "onescols": onescols,
        })

    res = run_bass_kernel_spmd(nc, in_maps, list(range(N_CORES)))
    acc = res.results[0]["outT"].astype(np.float64)
    for c in range(1, N_CORES):
        acc += res.results[c]["outT"]
    out = acc.T.astype(np.float32) + b_o[None, :]
    return out[None]  # [B, L, E]



# revision 3
# speedup vs baseline: 1.0885x; 1.0885x over previous
"""Trainium2 Bass kernel for nn_Attention_79645873537262.

Dense attention with per-head bias, key masking, sigmoid gate:
  t = x @ w_proj.T; per head: q,k,v
  a = softmax(scale*q@k.T + bias + mask); y = a@v
  y = sigmoid(x@w_g.T + b_g) * y;  out = y @ w_o.T + b_o

Sharding: tensor-parallel over heads, 2 heads per core on 8 cores.
Each core runs a fully independent program (no collectives): it computes
its 2 heads' attention plus its 128-column slice of the gate, and a
partial o_proj (contribution of its 128 y-columns to all 1024 outputs).
The host sums the 8 partial outputs and adds b_o (the "all-reduce").

On-device layout is transposed ("scores.T" flash style):
  scores.T[k,q] = kT.T@qT in PSUM; bias (pre-masked, pre-transposed,
  fp16) is added IN PLACE by the vector engine (PSUM += SBUF bias) --
  this keeps the PE out of the bias path entirely (the old ident@bias
  seeding cost 128 extra matmuls ~60us of PE time); exp on ScalarE
  (no max-subtraction: logits are ~N(0,2), |logit| < ~14 so exp is
  safe); y.T ext = [v | ones].T @ p gives y.T rows 0..63 and the
  softmax denominator in row 64. Normalization multiplies by a
  broadcast reciprocal (DMA DRAM round-trip to cross partitions).
All matmuls run in float32r.

Perf notes (from NTFF traces): PE streams 512-col matmuls at ~1.1
cols/ns (p-state MID) and HAM throttling halves that in windows, so
the only real lever is fewer matmul instructions and no PE idle.
Bias stream alternates Sync/GpSimd DGE queues; v transposes batch 4
per PSUM tile with 2 wide copies; per-q-half tails run one pass late;
within each q-half head 1 runs first so the final pass's norm chain
(head 0) skips the SBUF partition-shift DMA. Partial outputs are fp16
(halves the tail output-DMA drain; host accumulates in f64).
"""
import sys
import numpy as np
import ml_dtypes

try:
    import concourse.bass as bass
except ImportError:
    sys.path.insert(0, "/opt/trn_rl_repo")
    import concourse.bass as bass

import concourse.tile as tile
from concourse import bacc, mybir
from concourse.bass_utils import run_bass_kernel_spmd

B, L, E, H = 1, 2048, 1024, 16
HW = E // H                # 64
SCALE = HW ** -0.5
N_CORES = 8
HPC = H // N_CORES         # 2 heads per core
C2 = HPC * HW              # 128 y-columns per core
MASK_NEG = -60.0           # exp(-60 + max_bias) ~ 1e-23: dead keys vanish

f32 = mybir.dt.float32
f32r = mybir.dt.float32r
f16 = mybir.dt.float16

NE = E // 128              # 8 contraction chunks
NQ = L // 512              # 4 q-tiles of 512
NKT = L // 128             # 16 k-chunks of 128

_compiled = [None]
DEBUG = False


def _build():
    nc = bacc.Bacc("TRN2", target_bir_lowering=False, debug=False,
                   num_devices=N_CORES)

    xT_ap = nc.dram_tensor("xT", [E, L], f32r, kind="ExternalInput").ap()
    wpT_ap = nc.dram_tensor("wpT", [E, 3 * C2], f32r, kind="ExternalInput").ap()
    biasT_ap = nc.dram_tensor("biasT", [HPC, L, L], f16, kind="ExternalInput").ap()
    wgT_ap = nc.dram_tensor("wgT", [E, C2], f32r, kind="ExternalInput").ap()
    bgv_ap = nc.dram_tensor("bgv", [C2, 1], f32, kind="ExternalInput").ap()
    woT_ap = nc.dram_tensor("woT", [C2, E], f32r, kind="ExternalInput").ap()
    onescols_ap = nc.dram_tensor("onescols", [128, NKT], f32r, kind="ExternalInput").ap()
    identr_ap = nc.dram_tensor("identr", [128, 128], f32r, kind="ExternalInput").ap()
    outT_ap = nc.dram_tensor("outT", [E, L], f16, kind="ExternalOutput").ap()

    with tile.TileContext(nc) as tc:
        from contextlib import ExitStack
        with ExitStack() as ctx:
            pers = ctx.enter_context(tc.tile_pool(name="pers", bufs=1))
            work = ctx.enter_context(tc.tile_pool(name="work", bufs=1))
            biasp = ctx.enter_context(tc.tile_pool(name="bias", bufs=4))
            pp = ctx.enter_context(tc.tile_pool(name="pp", bufs=4))
            nrm = ctx.enter_context(tc.tile_pool(name="nrm", bufs=1))
            dramp = ctx.enter_context(tc.tile_pool(name="dram", bufs=4, space="DRAM"))
            outp = ctx.enter_context(tc.tile_pool(name="outp", bufs=3))
            # one PSUM layout for the whole kernel: no pool-transition barrier
            sp = ctx.enter_context(tc.tile_pool(name="s", bufs=2, space="PSUM"))
            yp = ctx.enter_context(tc.tile_pool(name="y", bufs=1, space="PSUM"))

            # --- proj-critical DMAs first (dispatch order matters) ---
            # x and w_proj arrive in per-chunk contiguous pieces so the proj
            # matmuls can start as soon as the first chunks land.
            xT_sb = [pers.tile([128, L], f32r, name=f"xT{e}", tag=f"xT{e}")
                     for e in range(NE)]
            wpT_sb = [pers.tile([128, 3 * C2], f32r, name=f"wpT{e}", tag=f"wpT{e}")
                      for e in range(NE)]
            for e in range(NE):
                nc.sync.dma_start(wpT_sb[e], wpT_ap[e * 128:(e + 1) * 128, :])
                nc.sync.dma_start(xT_sb[e][:, 0:1024],
                                  xT_ap[e * 128:(e + 1) * 128, 0:1024])
            for e in range(NE):
                nc.sync.dma_start(xT_sb[e][:, 1024:2048],
                                  xT_ap[e * 128:(e + 1) * 128, 1024:2048])
            wgT_sb = [pers.tile([128, C2], f32r, name=f"wgT{e}", tag=f"wgT{e}")
                      for e in range(NE)]
            for e in range(NE):
                nc.sync.dma_start(wgT_sb[e], wgT_ap[e * 128:(e + 1) * 128, :])
            bgv_sb = pers.tile([C2, 1], f32, tag="bgv")
            nc.sync.dma_start(bgv_sb, bgv_ap)
            woT_sb = pers.tile([C2, E], f32r, tag="woT")
            nc.sync.dma_start(woT_sb, woT_ap)
            identr_sb = pers.tile([128, 128], f32r, tag="identr")
            nc.sync.dma_start(identr_sb, identr_ap)
            # v tiles: [128 l, 130] per k-chunk: [v_h0 | ones | v_h1 | ones]
            v_all = pers.tile([128, NKT, 130], f32r, tag="v_all")
            nc.sync.dma_start(v_all[:, :, 64:65], onescols_ap.unsqueeze(2))
            nc.sync.dma_start(v_all[:, :, 129:130], onescols_ap.unsqueeze(2))

            q01 = pers.tile([128, L], f32r, tag="q01")
            k01 = pers.tile([128, L], f32r, tag="k01")
            g_sb = pers.tile([128, L], f32r, tag="g")
            ygT = pers.tile([128, L], f32r, tag="ygT")

            # ---------------- proj ----------------
            # e is the weight-change axis; the two inner 512-slices reuse the
            # loaded weight chunk (consecutive same-weight matmuls pipeline).
            vT01 = work.tile([128, L], f32r, tag="vT01")
            dests = [q01, k01, vT01]
            for lh in range(2):
                for f in range(3):
                    ps = sp.tile([128, 1024], f32, name=f"pj{f}_{lh}", tag="s")
                    for e in range(NE):
                        for ltq in range(2):
                            nc.tensor.matmul(
                                ps[:, ltq * 512:(ltq + 1) * 512],
                                wpT_sb[e][:, f * 128:(f + 1) * 128],
                                xT_sb[e][:, lh * 1024 + ltq * 512:
                                          lh * 1024 + (ltq + 1) * 512],
                                start=(e == 0), stop=(e == NE - 1))
                    nc.vector.tensor_copy(
                        dests[f][:, lh * 1024:(lh + 1) * 1024], ps)

            # transpose vT01 -> v_all[:, kt, :]; 4 transposes share one PSUM
            # tile so the PE never ping-pongs with the copy engine.
            for g4 in range(NKT // 4):
                ps = sp.tile([128, 4, 128], f32r, name=f"tr{g4}", tag="s")
                for i in range(4):
                    kt = g4 * 4 + i
                    nc.tensor.transpose(
                        ps[:, i, :], vT01[:, kt * 128:(kt + 1) * 128], identr_sb)
                nc.vector.tensor_copy(
                    v_all[:, g4 * 4:(g4 + 1) * 4, 0:64], ps[:, :, 0:64])
                nc.vector.tensor_copy(
                    v_all[:, g4 * 4:(g4 + 1) * 4, 65:129], ps[:, :, 64:128])

            # gate: g = sigmoid(wgT.T @ xT + bg)
            for lh in range(2):
                ps = sp.tile([C2, 1024], f32, name=f"pg{lh}", tag="s")
                for e in range(NE):
                    for ltq in range(2):
                        nc.tensor.matmul(
                            ps[:, ltq * 512:(ltq + 1) * 512], wgT_sb[e],
                            xT_sb[e][:, lh * 1024 + ltq * 512:
                                      lh * 1024 + (ltq + 1) * 512],
                            start=(e == 0), stop=(e == NE - 1))
                nc.scalar.activation(
                    g_sb[:, lh * 1024:(lh + 1) * 1024], ps,
                    mybir.ActivationFunctionType.Sigmoid,
                    bias=bgv_sb, scale=1.0)

            # ---------------- attention: 4 passes over (q-half, head) ----------------
            # y psum double-buffered across passes so pass p+1 accumulates
            # while pass p drains through its normalization chain. The
            # q-half tail (gate mul + o_proj) is emitted one pass late so the
            # PE stream never blocks on the normalization DMA round-trip.
            def attention_pass(qhalf, h):
                hb = h * 64
                y_ps = [yp.tile([65, 512], f32, name=f"y{qhalf}_{h}_{i}",
                                tag=f"y{i}", bufs=2) for i in range(2)]
                for kt in range(NKT):
                    bias_t = biasp.tile([128, 1024], f16,
                                        name=f"bias{qhalf}_{h}_{kt}", tag="bias")
                    dma_eng = nc.gpsimd if kt % 2 == 0 else nc.sync
                    dma_eng.dma_start(
                        bias_t, biasT_ap[h, kt * 128:(kt + 1) * 128,
                                         qhalf * 1024:(qhalf + 1) * 1024])
                    s_ps = sp.tile([128, 1024], f32,
                                   name=f"s{qhalf}_{h}_{kt}", tag="s")
                    for qq in range(2):
                        qs = qhalf * 1024 + qq * 512
                        nc.tensor.matmul(
                            s_ps[:, qq * 512:(qq + 1) * 512],
                            k01[hb:hb + 64, kt * 128:(kt + 1) * 128],
                            q01[hb:hb + 64, qs:qs + 512],
                            start=True, stop=True)
                    # bias add off the PE: PSUM += fp16 bias, in place
                    nc.vector.tensor_add(s_ps, s_ps, bias_t)
                    p_t = pp.tile([128, 1024], f32r,
                                  name=f"p{qhalf}_{h}_{kt}", tag="p")
                    nc.scalar.activation(
                        p_t, s_ps, mybir.ActivationFunctionType.Exp)
                    for qq in range(2):
                        nc.tensor.matmul(
                            y_ps[qq],
                            v_all[:, kt, h * 65:(h + 1) * 65],
                            p_t[:, qq * 512:(qq + 1) * 512],
                            start=(kt == 0), stop=(kt == NKT - 1))
                # normalization chains (softmax denominators in row 64)
                for qq in range(2):
                    qt = qhalf * 2 + qq
                    qsl = slice(qt * 512, (qt + 1) * 512)
                    sums_sb = nrm.tile([1, 512], f32,
                                       name=f"sums{qhalf}_{h}_{qq}", tag="sums")
                    nc.vector.tensor_copy(sums_sb, y_ps[qq][64:65, :])
                    dscr = dramp.tile([1, 512], f32,
                                      name=f"dscr{qhalf}_{h}_{qq}", tag="dscr")
                    nc.sync.dma_start(dscr, sums_sb)
                    sums_b = nrm.tile([64, 512], f32,
                                      name=f"sums_b{qhalf}_{h}_{qq}", tag="sums_b")
                    nc.sync.dma_start(sums_b, dscr.partition_broadcast(64))
                    rb_sb = nrm.tile([64, 512], f32, name=f"rb{qhalf}_{h}_{qq}", tag="rb")
                    nc.vector.reciprocal_approx_fast(rb_sb, sums_b)
                    if h == 0:
                        nc.vector.tensor_mul(
                            ygT[0:64, qsl], y_ps[qq][0:64, :], rb_sb)
                    else:
                        yg1 = nrm.tile([64, 512], f32r,
                                       name=f"yg1_{qhalf}_{qq}", tag="yg1")
                        nc.vector.tensor_mul(yg1, y_ps[qq][0:64, :], rb_sb)
                        nc.sync.dma_start(ygT[64:128, qsl], yg1)

            def qhalf_tail(qhalf):
                # gate multiply + o_proj partial for this q-half
                for qq in range(2):
                    qt = qhalf * 2 + qq
                    qsl = slice(qt * 512, (qt + 1) * 512)
                    nc.vector.tensor_mul(ygT[:, qsl], ygT[:, qsl], g_sb[:, qsl])
                for eo in range(NE):
                    ps = sp.tile([128, 1024], f32, name=f"po{qhalf}_{eo}", tag="s")
                    for qq in range(2):
                        qt = qhalf * 2 + qq
                        nc.tensor.matmul(
                            ps[:, qq * 512:(qq + 1) * 512],
                            woT_sb[:, eo * 128:(eo + 1) * 128],
                            ygT[:, qt * 512:(qt + 1) * 512],
                            start=True, stop=True)
                    ot = outp.tile([128, 1024], f16, name=f"ot{qhalf}_{eo}", tag="ot")
                    if eo % 2 == 0:
                        nc.vector.tensor_copy(ot, ps)
                    else:
                        nc.scalar.copy(ot, ps)
                    nc.sync.dma_start(
                        outT_ap[eo * 128:(eo + 1) * 128,
                                qhalf * 1024:(qhalf + 1) * 1024], ot)

            # head 1 first within each q-half: the final pass (head 0) has
            # the shift-free normalization chain, shortening the tail.
            attention_pass(0, 1)
            attention_pass(0, 0)
            attention_pass(1, 1)
            qhalf_tail(0)
            attention_pass(1, 0)
            qhalf_tail(1)

    nc.compile()
    return nc


def kernel(x, mask, bias, w_proj, w_o, b_o, w_g, b_g):
    x = np.asarray(x, dtype=np.float32)
    mask = np.asarray(mask)
    bias = np.asarray(bias, dtype=np.float32)
    w_proj = np.asarray(w_proj, dtype=np.float32)
    w_o = np.asarray(w_o, dtype=np.float32)
    b_o = np.asarray(b_o, dtype=np.float32)
    w_g = np.asarray(w_g, dtype=np.float32)
    b_g = np.asarray(b_g, dtype=np.float32)

    if _compiled[0] is None:
        _compiled[0] = _build()
    nc = _compiled[0]

    xT = np.ascontiguousarray(x[0].T)                      # [E, L]
    mask_add = np.where(mask[0], 0.0, MASK_NEG).astype(np.float32)  # [L]
    identr = np.eye(128, dtype=np.float32)
    onescols = np.ones((128, NKT), dtype=np.float32)

    in_maps = []
    for c in range(N_CORES):
        heads = [c * HPC + i for i in range(HPC)]
        wpT = np.empty((E, 3 * C2), dtype=np.float32)
        for i, h in enumerate(heads):
            r0 = h * 3 * HW
            wpT[:, 0 * C2 + i * HW: 0 * C2 + (i + 1) * HW] = \
                w_proj[r0: r0 + HW].T * SCALE               # q, pre-scaled
            wpT[:, 1 * C2 + i * HW: 1 * C2 + (i + 1) * HW] = \
                w_proj[r0 + HW: r0 + 2 * HW].T              # k
            wpT[:, 2 * C2 + i * HW: 2 * C2 + (i + 1) * HW] = \
                w_proj[r0 + 2 * HW: r0 + 3 * HW].T          # v
        biasT = np.ascontiguousarray(
            bias[0, :, :, heads].transpose(0, 2, 1))        # [2, Lk, Lq]
        biasT += mask_add[None, :, None]
        biasT = biasT.astype(np.float16)
        cols = slice(c * C2, (c + 1) * C2)
        wgT = np.ascontiguousarray(w_g[cols, :].T)          # [E, C2]
        bgv = np.ascontiguousarray(b_g[cols, None])         # [C2, 1]
        woT = np.ascontiguousarray(w_o[:, cols].T)          # [C2, E]
        in_maps.append({
            "xT": xT, "wpT": wpT, "biasT": biasT, "wgT": wgT,
            "bgv": bgv, "woT": woT, "identr": identr, "onescols": onescols,
        })

    res = run_bass_kernel_spmd(nc, in_maps, list(range(N_CORES)))
    acc = res.results[0]["outT"].astype(np.float64)
    for c in range(1, N_CORES):
        acc += res.results[c]["outT"]
    out = acc.T.astype(np.float32) + b_o[None, :]
    return out[None]  # [B, L, E]


# revision 10
# speedup vs baseline: 1.1246x; 1.0332x over previous
"""Trainium2 Bass kernel for nn_Attention_79645873537262.

Dense attention with per-head bias, key masking, sigmoid gate:
  t = x @ w_proj.T; per head: q,k,v
  a = softmax(scale*q@k.T + bias + mask); y = a@v
  y = sigmoid(x@w_g.T + b_g) * y;  out = y @ w_o.T + b_o

Sharding: tensor-parallel over heads, 2 heads per core on 8 cores.
Each core runs a fully independent program (no collectives): it computes
its 2 heads' attention plus its 128-column slice of the gate, and a
partial o_proj (contribution of its 128 y-columns to all 1024 outputs).
The host sums the 8 partial outputs and adds b_o (the "all-reduce").

On-device layout is transposed ("scores.T" flash style):
  scores.T[k,q] = kT.T@qT in PSUM; bias (pre-masked, pre-transposed,
  fp16) is added IN PLACE by the vector engine (PSUM += SBUF bias) --
  this keeps the PE out of the bias path entirely (the old ident@bias
  seeding cost 128 extra matmuls ~60us of PE time); exp on ScalarE
  (no max-subtraction: logits are ~N(0,2), |logit| < ~14 so exp is
  safe); y.T ext = [v | ones].T @ p gives y.T rows 0..63 and the
  softmax denominator in row 64. Normalization multiplies by a
  broadcast reciprocal (DMA DRAM round-trip to cross partitions).
All matmuls run in float32r.

Perf notes (from NTFF traces): PE streams 512-col matmuls at ~1.1
cols/ns (p-state MID) and HAM throttling halves that in windows, so
the only real lever is fewer matmul instructions and no PE idle.
Bias stream alternates Sync/GpSimd DGE queues; v transposes batch 4
per PSUM tile with 2 wide copies; per-q-half tails run one pass late;
within each q-half head 1 runs first so the final pass's norm chain
(head 0) skips the SBUF partition-shift DMA. Partial outputs are fp16
(halves the tail output-DMA drain; host accumulates in f64).
"""
import sys
import numpy as np
import ml_dtypes

try:
    import concourse.bass as bass
except ImportError:
    sys.path.insert(0, "/opt/trn_rl_repo")
    import concourse.bass as bass

import concourse.tile as tile
from concourse import bacc, mybir
from concourse.bass_utils import run_bass_kernel_spmd

B, L, E, H = 1, 2048, 1024, 16
HW = E // H                # 64
SCALE = HW ** -0.5
N_CORES = 8
HPC = H // N_CORES         # 2 heads per core
C2 = HPC * HW              # 128 y-columns per core
MASK_NEG = -60.0           # exp(-60 + max_bias) ~ 1e-23: dead keys vanish

f32 = mybir.dt.float32
f32r = mybir.dt.float32r
f16 = mybir.dt.float16

NE = E // 128              # 8 contraction chunks
NQ = L // 512              # 4 q-tiles of 512
NKT = L // 128             # 16 k-chunks of 128

_compiled = [None]
DEBUG = False


def _build():
    nc = bacc.Bacc("TRN2", target_bir_lowering=False, debug=False,
                   num_devices=N_CORES)

    xT_ap = nc.dram_tensor("xT", [E, L], f32r, kind="ExternalInput").ap()
    wpT_ap = nc.dram_tensor("wpT", [E, 3 * C2], f32r, kind="ExternalInput").ap()
    biasT_ap = nc.dram_tensor("biasT", [HPC, L, L], f16, kind="ExternalInput").ap()
    wgT_ap = nc.dram_tensor("wgT", [E, C2], f32r, kind="ExternalInput").ap()
    bgv_ap = nc.dram_tensor("bgv", [C2, 1], f32, kind="ExternalInput").ap()
    woT_ap = nc.dram_tensor("woT", [C2, E], f32r, kind="ExternalInput").ap()
    onescols_ap = nc.dram_tensor("onescols", [128, NKT], f32r, kind="ExternalInput").ap()
    identr_ap = nc.dram_tensor("identr", [128, 128], f32r, kind="ExternalInput").ap()
    outT_ap = nc.dram_tensor("outT", [E, L], f16, kind="ExternalOutput").ap()

    with tile.TileContext(nc) as tc:
        from contextlib import ExitStack
        with ExitStack() as ctx:
            pers = ctx.enter_context(tc.tile_pool(name="pers", bufs=1))
            work = ctx.enter_context(tc.tile_pool(name="work", bufs=1))
            biasp = ctx.enter_context(tc.tile_pool(name="bias", bufs=4))
            pp = ctx.enter_context(tc.tile_pool(name="pp", bufs=6))
            nrm = ctx.enter_context(tc.tile_pool(name="nrm", bufs=1))
            dramp = ctx.enter_context(tc.tile_pool(name="dram", bufs=4, space="DRAM"))
            outp = ctx.enter_context(tc.tile_pool(name="outp", bufs=3))
            # one PSUM layout for the whole kernel: no pool-transition barrier
            sp = ctx.enter_context(tc.tile_pool(name="s", bufs=2, space="PSUM"))
            yp = ctx.enter_context(tc.tile_pool(name="y", bufs=1, space="PSUM"))

            # --- proj-critical DMAs first (dispatch order matters) ---
            # x and w_proj arrive in per-chunk contiguous pieces so the proj
            # matmuls can start as soon as the first chunks land.
            xT_sb = [pers.tile([128, L], f32r, name=f"xT{e}", tag=f"xT{e}")
                     for e in range(NE)]
            wpT_sb = [pers.tile([128, 3 * C2], f32r, name=f"wpT{e}", tag=f"wpT{e}")
                      for e in range(NE)]
            for e in range(NE):
                nc.sync.dma_start(wpT_sb[e], wpT_ap[e * 128:(e + 1) * 128, :])
                nc.sync.dma_start(xT_sb[e][:, 0:1024],
                                  xT_ap[e * 128:(e + 1) * 128, 0:1024])
            for e in range(NE):
                nc.sync.dma_start(xT_sb[e][:, 1024:2048],
                                  xT_ap[e * 128:(e + 1) * 128, 1024:2048])
            wgT_sb = [pers.tile([128, C2], f32r, name=f"wgT{e}", tag=f"wgT{e}")
                      for e in range(NE)]
            for e in range(NE):
                nc.sync.dma_start(wgT_sb[e], wgT_ap[e * 128:(e + 1) * 128, :])
            bgv_sb = pers.tile([C2, 1], f32, tag="bgv")
            nc.sync.dma_start(bgv_sb, bgv_ap)
            woT_sb = pers.tile([C2, E], f32r, tag="woT")
            nc.sync.dma_start(woT_sb, woT_ap)
            identr_sb = pers.tile([128, 128], f32r, tag="identr")
            nc.sync.dma_start(identr_sb, identr_ap)
            # v tiles: [128 l, 130] per k-chunk: [v_h0 | ones | v_h1 | ones]
            v_all = pers.tile([128, NKT, 130], f32r, tag="v_all")
            nc.sync.dma_start(v_all[:, :, 64:65], onescols_ap.unsqueeze(2))
            nc.sync.dma_start(v_all[:, :, 129:130], onescols_ap.unsqueeze(2))

            q01 = pers.tile([128, L], f32r, tag="q01")
            k01 = pers.tile([128, L], f32r, tag="k01")
            g_sb = pers.tile([128, L], f32r, tag="g")
            ygT = pers.tile([128, L], f32r, tag="ygT")

            # ---------------- proj ----------------
            # e is the weight-change axis; the two inner 512-slices reuse the
            # loaded weight chunk (consecutive same-weight matmuls pipeline).
            vT01 = work.tile([128, L], f32r, tag="vT01")
            dests = [q01, k01, vT01]
            for lh in range(2):
                for f in range(3):
                    ps = sp.tile([128, 1024], f32, name=f"pj{f}_{lh}", tag="s")
                    for e in range(NE):
                        for ltq in range(2):
                            nc.tensor.matmul(
                                ps[:, ltq * 512:(ltq + 1) * 512],
                                wpT_sb[e][:, f * 128:(f + 1) * 128],
                                xT_sb[e][:, lh * 1024 + ltq * 512:
                                          lh * 1024 + (ltq + 1) * 512],
                                start=(e == 0), stop=(e == NE - 1))
                    nc.vector.tensor_copy(
                        dests[f][:, lh * 1024:(lh + 1) * 1024], ps)

            # gate: g = sigmoid(wgT.T @ xT + bg) -- before the transposes so
            # the PE stream stays dense while the vT01 copy lands.
            for lh in range(2):
                ps = sp.tile([C2, 1024], f32, name=f"pg{lh}", tag="s")
                for e in range(NE):
                    for ltq in range(2):
                        nc.tensor.matmul(
                            ps[:, ltq * 512:(ltq + 1) * 512], wgT_sb[e],
                            xT_sb[e][:, lh * 1024 + ltq * 512:
                                      lh * 1024 + (ltq + 1) * 512],
                            start=(e == 0), stop=(e == NE - 1))
                nc.scalar.activation(
                    g_sb[:, lh * 1024:(lh + 1) * 1024], ps,
                    mybir.ActivationFunctionType.Sigmoid,
                    bias=bgv_sb, scale=1.0)

            # transpose vT01 -> v_all[:, kt, :]; 4 transposes share one PSUM
            # tile so the PE never ping-pongs with the copy engine.
            for g4 in range(NKT // 4):
                ps = sp.tile([128, 4, 128], f32r, name=f"tr{g4}", tag="s")
                for i in range(4):
                    kt = g4 * 4 + i
                    nc.tensor.transpose(
                        ps[:, i, :], vT01[:, kt * 128:(kt + 1) * 128], identr_sb)
                nc.vector.tensor_copy(
                    v_all[:, g4 * 4:(g4 + 1) * 4, 0:64], ps[:, :, 0:64])
                nc.vector.tensor_copy(
                    v_all[:, g4 * 4:(g4 + 1) * 4, 65:129], ps[:, :, 64:128])

            # ---------------- attention: 4 passes over (q-half, head) ----------------
            # y psum double-buffered across passes so pass p+1 accumulates
            # while pass p drains through its normalization chain. The
            # q-half tail (gate mul + o_proj) is emitted one pass late so the
            # PE stream never blocks on the normalization DMA round-trip.
            # pv matmuls run LOOK k-steps behind qk so the PE never waits on
            # the add->exp chain (PE idle gaps re-throttle HAM to 1.2 GHz).
            # Each pass's normalization chains and the previous q-half's
            # tail are emitted a few steps into the NEXT pass (y psum is
            # double-buffered across passes); the injections are spread so
            # no single vector-queue bubble exceeds the pv lookahead slack.
            LOOK = 4

            def norm_chains(qhalf, h, y_ps):
                # normalization chains (softmax denominators in row 64)
                for qq in range(2):
                    qt = qhalf * 2 + qq
                    qsl = slice(qt * 512, (qt + 1) * 512)
                    sums_sb = nrm.tile([1, 512], f32,
                                       name=f"sums{qhalf}_{h}_{qq}", tag="sums")
                    nc.vector.tensor_copy(sums_sb, y_ps[qq][64:65, :])
                    dscr = dramp.tile([1, 512], f32,
                                      name=f"dscr{qhalf}_{h}_{qq}", tag="dscr")
                    nc.sync.dma_start(dscr, sums_sb)
                    sums_b = nrm.tile([64, 512], f32,
                                      name=f"sums_b{qhalf}_{h}_{qq}", tag="sums_b")
                    nc.sync.dma_start(sums_b, dscr.partition_broadcast(64))
                    rb_sb = nrm.tile([64, 512], f32, name=f"rb{qhalf}_{h}_{qq}", tag="rb")
                    nc.vector.reciprocal_approx_fast(rb_sb, sums_b)
                    if h == 0:
                        nc.vector.tensor_mul(
                            ygT[0:64, qsl], y_ps[qq][0:64, :], rb_sb)
                    else:
                        yg1 = nrm.tile([64, 512], f32r,
                                       name=f"yg1_{qhalf}_{qq}", tag="yg1")
                        nc.vector.tensor_mul(yg1, y_ps[qq][0:64, :], rb_sb)
                        nc.sync.dma_start(ygT[64:128, qsl], yg1)

            def attention_pass(qhalf, h, pending=()):
                hb = h * 64
                y_ps = [yp.tile([65, 512], f32, name=f"y{qhalf}_{h}_{i}",
                                tag=f"y{i}", bufs=2) for i in range(2)]
                pend = dict(pending)
                pqueue = []
                for kt in range(NKT + LOOK):
                    if kt < NKT:
                        bias_t = biasp.tile([128, 1024], f16,
                                            name=f"bias{qhalf}_{h}_{kt}", tag="bias")
                        dma_eng = nc.gpsimd if kt % 2 == 0 else nc.sync
                        dma_eng.dma_start(
                            bias_t, biasT_ap[h, kt * 128:(kt + 1) * 128,
                                             qhalf * 1024:(qhalf + 1) * 1024])
                        s_ps = sp.tile([128, 1024], f32,
                                       name=f"s{qhalf}_{h}_{kt}", tag="s")
                        for qq in range(2):
                            qs = qhalf * 1024 + qq * 512
                            nc.tensor.matmul(
                                s_ps[:, qq * 512:(qq + 1) * 512],
                                k01[hb:hb + 64, kt * 128:(kt + 1) * 128],
                                q01[hb:hb + 64, qs:qs + 512],
                                start=True, stop=True)
                        # bias add off the PE: PSUM += fp16 bias, in place
                        nc.vector.tensor_add(s_ps, s_ps, bias_t)
                        p_t = pp.tile([128, 1024], f32r,
                                      name=f"p{qhalf}_{h}_{kt}", tag="p")
                        nc.scalar.activation(
                            p_t, s_ps, mybir.ActivationFunctionType.Exp)
                        pqueue.append((kt, p_t))
                    if kt >= LOOK:
                        pkt, p_t = pqueue[kt - LOOK]
                        for qq in range(2):
                            nc.tensor.matmul(
                                y_ps[qq],
                                v_all[:, pkt, h * 65:(h + 1) * 65],
                                p_t[:, qq * 512:(qq + 1) * 512],
                                start=(pkt == 0), stop=(pkt == NKT - 1))
                    if kt in pend:
                        pend.pop(kt)()
                return lambda: norm_chains(qhalf, h, y_ps)

            def qhalf_tail(qhalf, eo_range, gate=False):
                # gate multiply + o_proj partial for this q-half
                if gate:
                    for qq in range(2):
                        qt = qhalf * 2 + qq
                        qsl = slice(qt * 512, (qt + 1) * 512)
                        nc.vector.tensor_mul(ygT[:, qsl], ygT[:, qsl], g_sb[:, qsl])
                for eo in eo_range:
                    ps = sp.tile([128, 1024], f32, name=f"po{qhalf}_{eo}", tag="s")
                    for qq in range(2):
                        qt = qhalf * 2 + qq
                        nc.tensor.matmul(
                            ps[:, qq * 512:(qq + 1) * 512],
                            woT_sb[:, eo * 128:(eo + 1) * 128],
                            ygT[:, qt * 512:(qt + 1) * 512],
                            start=True, stop=True)
                    ot = outp.tile([128, 1024], f16, name=f"ot{qhalf}_{eo}", tag="ot")
                    if eo % 2 == 0:
                        nc.vector.tensor_copy(ot, ps)
                    else:
                        nc.scalar.copy(ot, ps)
                    nc.sync.dma_start(
                        outT_ap[eo * 128:(eo + 1) * 128,
                                qhalf * 1024:(qhalf + 1) * 1024], ot)

            # head 1 first within each q-half: the final pass (head 0) has
            # the shift-free normalization chain, shortening the tail.
            # Pass P's norm chains are injected early into pass P+1, and the
            # first q-half's tail is split across two injection points so no
            # vector-queue bubble outruns the pv lookahead slack.
            c01 = attention_pass(0, 1)
            c00 = attention_pass(0, 0, pending={1: c01})
            c11 = attention_pass(1, 1, pending={1: c00})
            c10 = attention_pass(1, 0, pending={
                1: c11,
                5: lambda: qhalf_tail(0, range(0, 4), gate=True),
                9: lambda: qhalf_tail(0, range(4, 8)),
            })
            c10()
            qhalf_tail(1, range(NE), gate=True)

    nc.compile()
    return nc


def kernel(x, mask, bias, w_proj, w_o, b_o, w_g, b_g):
    x = np.asarray(x, dtype=np.float32)
    mask = np.asarray(mask)
    bias = np.asarray(bias, dtype=np.float32)
    w_proj = np.asarray(w_proj, dtype=np.float32)
    w_o = np.asarray(w_o, dtype=np.float32)
    b_o = np.asarray(b_o, dtype=np.float32)
    w_g = np.asarray(w_g, dtype=np.float32)
    b_g = np.asarray(b_g, dtype=np.float32)

    if _compiled[0] is None:
        _compiled[0] = _build()
    nc = _compiled[0]

    xT = np.ascontiguousarray(x[0].T)                      # [E, L]
    mask_add = np.where(mask[0], 0.0, MASK_NEG).astype(np.float32)  # [L]
    identr = np.eye(128, dtype=np.float32)
    onescols = np.ones((128, NKT), dtype=np.float32)

    in_maps = []
    for c in range(N_CORES):
        heads = [c * HPC + i for i in range(HPC)]
        wpT = np.empty((E, 3 * C2), dtype=np.float32)
        for i, h in enumerate(heads):
            r0 = h * 3 * HW
            wpT[:, 0 * C2 + i * HW: 0 * C2 + (i + 1) * HW] = \
                w_proj[r0: r0 + HW].T * SCALE               # q, pre-scaled
            wpT[:, 1 * C2 + i * HW: 1 * C2 + (i + 1) * HW] = \
                w_proj[r0 + HW: r0 + 2 * HW].T              # k
            wpT[:, 2 * C2 + i * HW: 2 * C2 + (i + 1) * HW] = \
                w_proj[r0 + 2 * HW: r0 + 3 * HW].T          # v
        biasT = np.ascontiguousarray(
            bias[0, :, :, heads].transpose(0, 2, 1))        # [2, Lk, Lq]
        biasT += mask_add[None, :, None]
        biasT = biasT.astype(np.float16)
        cols = slice(c * C2, (c + 1) * C2)
        wgT = np.ascontiguousarray(w_g[cols, :].T)          # [E, C2]
        bgv = np.ascontiguousarray(b_g[cols, None])         # [C2, 1]
        woT = np.ascontiguousarray(w_o[:, cols].T)          # [C2, E]
        in_maps.append({
            "xT": xT, "wpT": wpT, "biasT": biasT, "wgT": wgT,
            "bgv": bgv, "woT": woT, "identr": identr, "onescols": onescols,
        })

    res = run_bass_kernel_spmd(nc, in_maps, list(range(N_CORES)))
    acc = res.results[0]["outT"].astype(np.float64)
    for c in range(1, N_CORES):
        acc += res.results[c]["outT"]
    out = acc.T.astype(np.float32) + b_o[None, :]
    return out[None]  # [B, L, E]


# revision 29
# speedup vs baseline: 1.1990x; 1.0661x over previous
"""Trainium2 Bass kernel for nn_Attention_79645873537262.

Dense attention with per-head bias, key masking, sigmoid gate:
  t = x @ w_proj.T; per head: q,k,v
  a = softmax(scale*q@k.T + bias + mask); y = a@v
  y = sigmoid(x@w_g.T + b_g) * y;  out = y @ w_o.T + b_o

Sharding: tensor-parallel over heads, 2 heads per core on 8 cores.
Each core runs a fully independent program (no collectives): it computes
its 2 heads' attention plus its 128-column slice of the gate, and a
partial o_proj (contribution of its 128 y-columns to all 1024 outputs).
The host sums the 8 partial outputs and adds b_o (the "all-reduce").

On-device layout is transposed ("scores.T" flash style):
  scores.T[k,q] = kT.T@qT in PSUM; bias (pre-masked, pre-transposed,
  fp16) is added IN PLACE by the vector engine (PSUM += SBUF bias) --
  this keeps the PE out of the bias path entirely (the old ident@bias
  seeding cost 128 extra matmuls ~60us of PE time); exp on ScalarE
  (no max-subtraction: logits are ~N(0,2), |logit| < ~14 so exp is
  safe); y.T ext = [v | ones].T @ p gives y.T rows 0..63 and the
  softmax denominator in row 64. Normalization multiplies by a
  broadcast reciprocal (DMA DRAM round-trip to cross partitions).
All matmuls run in float32r.

Perf notes (from NTFF traces): PE streams 512-col matmuls at ~1.1
cols/ns (p-state MID) and HAM throttling halves that in windows, so
the only real lever is fewer matmul instructions and no PE idle.
Bias stream alternates Sync/GpSimd DGE queues; v transposes batch 4
per PSUM tile with 2 wide copies; per-q-half tails run one pass late;
within each q-half head 1 runs first so the final pass's norm chain
(head 0) skips the SBUF partition-shift DMA. Partial outputs are fp16
(halves the tail output-DMA drain; host accumulates in f64).
"""
import sys
import numpy as np
import ml_dtypes

try:
    import concourse.bass as bass
except ImportError:
    sys.path.insert(0, "/opt/trn_rl_repo")
    import concourse.bass as bass

import concourse.tile as tile
from concourse import bacc, mybir
from concourse.bass_utils import run_bass_kernel_spmd

B, L, E, H = 1, 2048, 1024, 16
HW = E // H                # 64
SCALE = HW ** -0.5
N_CORES = 8
HPC = H // N_CORES         # 2 heads per core
C2 = HPC * HW              # 128 y-columns per core
MASK_NEG = -60.0           # exp(-60 + max_bias) ~ 1e-23: dead keys vanish
EXPSHIFT = -7.0            # max observed logit 15.06 -> exp(8.06) fits fp16

f32 = mybir.dt.float32
f32r = mybir.dt.float32r
f16 = mybir.dt.float16

NE = E // 128              # 8 contraction chunks
NQ = L // 512              # 4 q-tiles of 512
NKT = L // 128             # 16 k-chunks of 128

_compiled = [None]
DEBUG = False


def _build():
    nc = bacc.Bacc("TRN2", target_bir_lowering=False, debug=False,
                   num_devices=N_CORES)

    xT_ap = nc.dram_tensor("xT", [E, L], f32r, kind="ExternalInput").ap()
    wpT_ap = nc.dram_tensor("wpT", [E, 3 * C2], f32r, kind="ExternalInput").ap()
    biasT_ap = nc.dram_tensor("biasT", [HPC, L, L], f16, kind="ExternalInput").ap()
    wgT_ap = nc.dram_tensor("wgT", [E, C2], f32r, kind="ExternalInput").ap()
    bgv_ap = nc.dram_tensor("bgv", [C2, 1], f32, kind="ExternalInput").ap()
    woT_ap = nc.dram_tensor("woT", [C2, E], f16, kind="ExternalInput").ap()
    onescols_ap = nc.dram_tensor("onescols", [128, NKT], f16, kind="ExternalInput").ap()
    identh_ap = nc.dram_tensor("identh", [128, 128], f16, kind="ExternalInput").ap()
    outT_ap = nc.dram_tensor("outT", [E, L], f16, kind="ExternalOutput").ap()

    with tile.TileContext(nc) as tc:
        from contextlib import ExitStack
        with ExitStack() as ctx:
            pers = ctx.enter_context(tc.tile_pool(name="pers", bufs=1))
            work = ctx.enter_context(tc.tile_pool(name="work", bufs=1))
            biasp = ctx.enter_context(tc.tile_pool(name="bias", bufs=4))
            pp = ctx.enter_context(tc.tile_pool(name="pp", bufs=6))
            nrm = ctx.enter_context(tc.tile_pool(name="nrm", bufs=1))
            dramp = ctx.enter_context(tc.tile_pool(name="dram", bufs=4, space="DRAM"))
            outp = ctx.enter_context(tc.tile_pool(name="outp", bufs=3))
            # one PSUM layout for the whole kernel: no pool-transition barrier
            sp = ctx.enter_context(tc.tile_pool(name="s", bufs=2, space="PSUM"))
            yp = ctx.enter_context(tc.tile_pool(name="y", bufs=1, space="PSUM"))

            # --- proj-critical DMAs first (dispatch order matters) ---
            # x and w_proj arrive in per-chunk contiguous pieces so the proj
            # matmuls can start as soon as the first chunks land.
            xT_sb = [pers.tile([128, L], f32r, name=f"xT{e}", tag=f"xT{e}")
                     for e in range(NE)]
            wpT_sb = [pers.tile([128, 3 * C2], f32r, name=f"wpT{e}", tag=f"wpT{e}")
                      for e in range(NE)]
            for e in range(NE):
                nc.sync.dma_start(wpT_sb[e], wpT_ap[e * 128:(e + 1) * 128, :])
                nc.sync.dma_start(xT_sb[e][:, 0:1024],
                                  xT_ap[e * 128:(e + 1) * 128, 0:1024])
            for e in range(NE):
                nc.sync.dma_start(xT_sb[e][:, 1024:2048],
                                  xT_ap[e * 128:(e + 1) * 128, 1024:2048])
            wgT_sb = [pers.tile([128, C2], f32r, name=f"wgT{e}", tag=f"wgT{e}")
                      for e in range(NE)]
            for e in range(NE):
                nc.sync.dma_start(wgT_sb[e], wgT_ap[e * 128:(e + 1) * 128, :])
            bgv_sb = pers.tile([C2, 1], f32, tag="bgv")
            nc.sync.dma_start(bgv_sb, bgv_ap)
            woT_sb = pers.tile([C2, E], f16, tag="woT")
            nc.sync.dma_start(woT_sb, woT_ap)
            identh_sb = pers.tile([128, 128], f16, tag="identh")
            nc.sync.dma_start(identh_sb, identh_ap)
            # v tiles: [128 l, 130] per k-chunk: [v_h0 | ones | v_h1 | ones]
            v_all = pers.tile([128, NKT, 130], f16, tag="v_all")
            nc.sync.dma_start(v_all[:, :, 64:65], onescols_ap.unsqueeze(2))
            nc.sync.dma_start(v_all[:, :, 129:130], onescols_ap.unsqueeze(2))

            q01 = pers.tile([128, L], f32r, tag="q01")
            k01 = pers.tile([128, L], f32r, tag="k01")
            g_sb = pers.tile([128, L], f16, tag="g")
            ygT = pers.tile([128, L], f16, tag="ygT")
            # uniform exp shift: keeps max p = exp(15.06 + EXPSHIFT) well
            # inside fp16 range; denominators scale identically, so softmax
            # ratios are exact.
            nbias = pers.tile([128, 1], f32, tag="nbias")
            nc.vector.memset(nbias, EXPSHIFT)

            # ---------------- proj ----------------
            # e is the weight-change axis; the two inner 512-slices reuse the
            # loaded weight chunk (consecutive same-weight matmuls pipeline).
            vT01 = work.tile([128, L], f16, tag="vT01")
            dests = [q01, k01, vT01]
            for lh in range(2):
                for f in range(3):
                    ps = sp.tile([128, 1024], f32, name=f"pj{f}_{lh}", tag="s")
                    for e in range(NE):
                        for ltq in range(2):
                            nc.tensor.matmul(
                                ps[:, ltq * 512:(ltq + 1) * 512],
                                wpT_sb[e][:, f * 128:(f + 1) * 128],
                                xT_sb[e][:, lh * 1024 + ltq * 512:
                                          lh * 1024 + (ltq + 1) * 512],
                                start=(e == 0), stop=(e == NE - 1))
                    nc.vector.tensor_copy(
                        dests[f][:, lh * 1024:(lh + 1) * 1024], ps)

            # gate: g = sigmoid(wgT.T @ xT + bg) -- before the transposes so
            # the PE stream stays dense while the vT01 copy lands.
            for lh in range(2):
                ps = sp.tile([C2, 1024], f32, name=f"pg{lh}", tag="s")
                for e in range(NE):
                    for ltq in range(2):
                        nc.tensor.matmul(
                            ps[:, ltq * 512:(ltq + 1) * 512], wgT_sb[e],
                            xT_sb[e][:, lh * 1024 + ltq * 512:
                                      lh * 1024 + (ltq + 1) * 512],
                            start=(e == 0), stop=(e == NE - 1))
                nc.scalar.activation(
                    g_sb[:, lh * 1024:(lh + 1) * 1024], ps,
                    mybir.ActivationFunctionType.Sigmoid,
                    bias=bgv_sb, scale=1.0)

            # transpose vT01 -> v_all[:, kt, :]; 4 fp16 transposes share one
            # PSUM tile so the PE never ping-pongs with the copy engine.
            for g4 in range(NKT // 4):
                ps = sp.tile([128, 4, 128], f16, name=f"tr{g4}", tag="s")
                for i in range(4):
                    kt = g4 * 4 + i
                    nc.tensor.transpose(
                        ps[:, i, :], vT01[:, kt * 128:(kt + 1) * 128], identh_sb)
                nc.vector.tensor_copy(
                    v_all[:, g4 * 4:(g4 + 1) * 4, 0:64], ps[:, :, 0:64])
                nc.vector.tensor_copy(
                    v_all[:, g4 * 4:(g4 + 1) * 4, 65:129], ps[:, :, 64:128])

            # ---------------- attention: 4 passes over (q-half, head) ----------------
            # y psum double-buffered across passes so pass p+1 accumulates
            # while pass p drains through its normalization chain. The
            # q-half tail (gate mul + o_proj) is emitted one pass late so the
            # PE stream never blocks on the normalization DMA round-trip.
            # pv matmuls run LOOK k-steps behind qk so the PE never waits on
            # the add->exp chain (PE idle gaps re-throttle HAM to 1.2 GHz).
            # Each pass's normalization chains and the previous q-half's
            # tail are emitted a few steps into the NEXT pass (y psum is
            # double-buffered across passes); the injections are spread so
            # no single vector-queue bubble exceeds the pv lookahead slack.
            LOOK = 4

            def norm_chains(qhalf, h, y_ps):
                # normalization chains (softmax denominators in row 64)
                for qq in range(2):
                    qt = qhalf * 2 + qq
                    qsl = slice(qt * 512, (qt + 1) * 512)
                    sums_sb = nrm.tile([1, 512], f32,
                                       name=f"sums{qhalf}_{h}_{qq}", tag="sums")
                    nc.vector.tensor_copy(sums_sb, y_ps[qq][64:65, :])
                    dscr = dramp.tile([1, 512], f32,
                                      name=f"dscr{qhalf}_{h}_{qq}", tag="dscr")
                    nc.sync.dma_start(dscr, sums_sb)
                    sums_b = nrm.tile([64, 512], f32,
                                      name=f"sums_b{qhalf}_{h}_{qq}", tag="sums_b")
                    nc.sync.dma_start(sums_b, dscr.partition_broadcast(64))
                    rb_sb = nrm.tile([64, 512], f32, name=f"rb{qhalf}_{h}_{qq}", tag="rb")
                    nc.vector.reciprocal_approx_fast(rb_sb, sums_b)
                    if h == 0:
                        nc.vector.tensor_mul(
                            ygT[0:64, qsl], y_ps[qq][0:64, :], rb_sb)
                    else:
                        yg1 = nrm.tile([64, 512], f16,
                                       name=f"yg1_{qhalf}_{qq}", tag="yg1")
                        nc.vector.tensor_mul(yg1, y_ps[qq][0:64, :], rb_sb)
                        nc.sync.dma_start(ygT[64:128, qsl], yg1)

            def attention_pass(qhalf, h, pending=()):
                hb = h * 64
                y_ps = [yp.tile([65, 512], f32, name=f"y{qhalf}_{h}_{i}",
                                tag=f"y{i}", bufs=2) for i in range(2)]
                pend = dict(pending)
                pqueue = []
                for kt in range(NKT + LOOK):
                    if kt < NKT:
                        bias_t = biasp.tile([128, 1024], f16,
                                            name=f"bias{qhalf}_{h}_{kt}", tag="bias")
                        dma_eng = nc.gpsimd if kt % 2 == 0 else nc.sync
                        dma_eng.dma_start(
                            bias_t, biasT_ap[h, kt * 128:(kt + 1) * 128,
                                             qhalf * 1024:(qhalf + 1) * 1024])
                        s_ps = sp.tile([128, 1024], f32,
                                       name=f"s{qhalf}_{h}_{kt}", tag="s")
                        for qq in range(2):
                            qs = qhalf * 1024 + qq * 512
                            nc.tensor.matmul(
                                s_ps[:, qq * 512:(qq + 1) * 512],
                                k01[hb:hb + 64, kt * 128:(kt + 1) * 128],
                                q01[hb:hb + 64, qs:qs + 512],
                                start=True, stop=True)
                        # bias add off the PE: PSUM += fp16 bias, in place
                        nc.vector.tensor_add(s_ps, s_ps, bias_t)
                        # exp to fp16 with a uniform e^-2 scale for range
                        # margin (denominator scales identically: exact)
                        p_t = pp.tile([128, 1024], f16,
                                      name=f"p{qhalf}_{h}_{kt}", tag="p")
                        nc.scalar.activation(
                            p_t, s_ps, mybir.ActivationFunctionType.Exp,
                            bias=nbias)
                        pqueue.append((kt, p_t))
                    if kt >= LOOK:
                        pkt, p_t = pqueue[kt - LOOK]
                        for qq in range(2):
                            nc.tensor.matmul(
                                y_ps[qq],
                                v_all[:, pkt, h * 65:(h + 1) * 65],
                                p_t[:, qq * 512:(qq + 1) * 512],
                                start=(pkt == 0), stop=(pkt == NKT - 1))
                    if kt in pend:
                        pend.pop(kt)()
                return lambda: norm_chains(qhalf, h, y_ps)

            def qhalf_tail(qhalf, eo_range, gate=False):
                # gate multiply + o_proj partial for this q-half
                if gate:
                    for qq in range(2):
                        qt = qhalf * 2 + qq
                        qsl = slice(qt * 512, (qt + 1) * 512)
                        nc.vector.tensor_mul(ygT[:, qsl], ygT[:, qsl], g_sb[:, qsl])
                for eo in eo_range:
                    ps = sp.tile([128, 1024], f32, name=f"po{qhalf}_{eo}", tag="s")
                    for qq in range(2):
                        qt = qhalf * 2 + qq
                        nc.tensor.matmul(
                            ps[:, qq * 512:(qq + 1) * 512],
                            woT_sb[:, eo * 128:(eo + 1) * 128],
                            ygT[:, qt * 512:(qt + 1) * 512],
                            start=True, stop=True)
                    ot = outp.tile([128, 1024], f16, name=f"ot{qhalf}_{eo}", tag="ot")
                    if eo % 2 == 0:
                        nc.vector.tensor_copy(ot, ps)
                    else:
                        nc.scalar.copy(ot, ps)
                    nc.sync.dma_start(
                        outT_ap[eo * 128:(eo + 1) * 128,
                                qhalf * 1024:(qhalf + 1) * 1024], ot)

            # head 1 first within each q-half: the final pass (head 0) has
            # the shift-free normalization chain, shortening the tail.
            # Pass P's norm chains are injected early into pass P+1, and the
            # first q-half's tail is split across two injection points so no
            # vector-queue bubble outruns the pv lookahead slack.
            c01 = attention_pass(0, 1)
            c00 = attention_pass(0, 0, pending={1: c01})
            c11 = attention_pass(1, 1, pending={1: c00})
            c10 = attention_pass(1, 0, pending={
                1: c11,
                5: lambda: qhalf_tail(0, range(0, 4), gate=True),
                9: lambda: qhalf_tail(0, range(4, 8)),
            })
            c10()
            qhalf_tail(1, range(NE), gate=True)

    nc.compile()
    return nc


def kernel(x, mask, bias, w_proj, w_o, b_o, w_g, b_g):
    x = np.asarray(x, dtype=np.float32)
    mask = np.asarray(mask)
    bias = np.asarray(bias, dtype=np.float32)
    w_proj = np.asarray(w_proj, dtype=np.float32)
    w_o = np.asarray(w_o, dtype=np.float32)
    b_o = np.asarray(b_o, dtype=np.float32)
    w_g = np.asarray(w_g, dtype=np.float32)
    b_g = np.asarray(b_g, dtype=np.float32)

    if _compiled[0] is None:
        _compiled[0] = _build()
    nc = _compiled[0]

    xT = np.ascontiguousarray(x[0].T)                      # [E, L]
    mask_add = np.where(mask[0], 0.0, MASK_NEG).astype(np.float32)  # [L]
    onescols = np.ones((128, NKT), dtype=np.float16)
    identh = np.eye(128, dtype=np.float16)

    in_maps = []
    for c in range(N_CORES):
        heads = [c * HPC + i for i in range(HPC)]
        wpT = np.empty((E, 3 * C2), dtype=np.float32)
        for i, h in enumerate(heads):
            r0 = h * 3 * HW
            wpT[:, 0 * C2 + i * HW: 0 * C2 + (i + 1) * HW] = \
                w_proj[r0: r0 + HW].T * SCALE               # q, pre-scaled
            wpT[:, 1 * C2 + i * HW: 1 * C2 + (i + 1) * HW] = \
                w_proj[r0 + HW: r0 + 2 * HW].T              # k
            wpT[:, 2 * C2 + i * HW: 2 * C2 + (i + 1) * HW] = \
                w_proj[r0 + 2 * HW: r0 + 3 * HW].T          # v
        biasT = np.ascontiguousarray(
            bias[0, :, :, heads].transpose(0, 2, 1))        # [2, Lk, Lq]
        biasT += mask_add[None, :, None]
        biasT = biasT.astype(np.float16)
        cols = slice(c * C2, (c + 1) * C2)
        wgT = np.ascontiguousarray(w_g[cols, :].T)          # [E, C2]
        bgv = np.ascontiguousarray(b_g[cols, None])         # [C2, 1]
        woT = np.ascontiguousarray(w_o[:, cols].T).astype(np.float16)  # [C2, E]
        in_maps.append({
            "xT": xT, "wpT": wpT, "biasT": biasT, "wgT": wgT,
            "bgv": bgv, "woT": woT, "onescols": onescols, "identh": identh,
        })

    res = run_bass_kernel_spmd(nc, in_maps, list(range(N_CORES)))
    acc = res.results[0]["outT"].astype(np.float64)
    for c in range(1, N_CORES):
        acc += res.results[c]["outT"]
    out = acc.T.astype(np.float32) + b_o[None, :]
    return out[None]  # [B, L, E]


# revision 44
# speedup vs baseline: 1.3588x; 1.1333x over previous
"""Trainium2 Bass kernel for nn_Attention_79645873537262.

Dense attention with per-head bias, key masking, sigmoid gate:
  t = x @ w_proj.T; per head: q,k,v
  a = softmax(scale*q@k.T + bias + mask); y = a@v
  y = sigmoid(x@w_g.T + b_g) * y;  out = y @ w_o.T + b_o

Sharding: tensor-parallel over heads, 2 heads per core on 8 cores.
Each core runs a fully independent program (no collectives): it computes
its 2 heads' attention plus its 128-column slice of the gate, and a
partial o_proj (contribution of its 128 y-columns to all 1024 outputs).
The host sums the 8 partial outputs and adds b_o (the "all-reduce").

On-device layout is transposed ("scores.T" flash style):
  scores.T[k,q] = kT.T@qT in PSUM; bias (pre-masked, pre-transposed,
  fp16) is added IN PLACE by the vector engine (PSUM += SBUF bias) --
  this keeps the PE out of the bias path entirely (the old ident@bias
  seeding cost 128 extra matmuls ~60us of PE time); exp on ScalarE
  (no max-subtraction: logits are ~N(0,2), |logit| < ~14 so exp is
  safe); y.T ext = [v | ones].T @ p gives y.T rows 0..63 and the
  softmax denominator in row 64. Normalization multiplies by a
  broadcast reciprocal (DMA DRAM round-trip to cross partitions).
All matmuls run in float32r.

Perf notes (from NTFF traces): PE streams 512-col matmuls at ~1.1
cols/ns (p-state MID) and HAM throttling halves that in windows, so
the only real lever is fewer matmul instructions and no PE idle.
Bias stream alternates Sync/GpSimd DGE queues; v transposes batch 4
per PSUM tile with 2 wide copies; per-q-half tails run one pass late;
within each q-half head 1 runs first so the final pass's norm chain
(head 0) skips the SBUF partition-shift DMA. Partial outputs are fp16
(halves the tail output-DMA drain; host accumulates in f64).
"""
import sys
import numpy as np
import ml_dtypes

try:
    import concourse.bass as bass
except ImportError:
    sys.path.insert(0, "/opt/trn_rl_repo")
    import concourse.bass as bass

import concourse.tile as tile
from concourse import bacc, mybir
from concourse.bass_utils import run_bass_kernel_spmd

B, L, E, H = 1, 2048, 1024, 16
HW = E // H                # 64
SCALE = HW ** -0.5
N_CORES = 8
HPC = H // N_CORES         # 2 heads per core
C2 = HPC * HW              # 128 y-columns per core
MASK_NEG = -60.0           # exp(-60 + max_bias) ~ 1e-23: dead keys vanish
# p = exp(s + DEVSHIFT) * exp(bias + HOSTSHIFT): the total e^-7 shift keeps
# max p = exp(15.06 - 7) in fp16; the device part keeps exp(s_max=13.43 - 4)
# in fp16 too. Denominators scale identically, so softmax ratios are exact.
DEVSHIFT = -4.0
HOSTSHIFT = -3.0

f32 = mybir.dt.float32
f32r = mybir.dt.float32r
f16 = mybir.dt.float16

NE = E // 128              # 8 contraction chunks
NQ = L // 512              # 4 q-tiles of 512
NKT = L // 128             # 16 k-chunks of 128

_compiled = [None]
DEBUG = False


def _build():
    nc = bacc.Bacc("TRN2", target_bir_lowering=False, debug=False,
                   num_devices=N_CORES)

    xT_ap = nc.dram_tensor("xT", [E, L], f32r, kind="ExternalInput").ap()
    wpT_ap = nc.dram_tensor("wpT", [E, 3 * C2], f32r, kind="ExternalInput").ap()
    biasT_ap = nc.dram_tensor("biasT", [HPC, L, L], f16, kind="ExternalInput").ap()
    wgT_ap = nc.dram_tensor("wgT", [E, C2], f32r, kind="ExternalInput").ap()
    bgv_ap = nc.dram_tensor("bgv", [C2, 1], f32, kind="ExternalInput").ap()
    woT_ap = nc.dram_tensor("woT", [C2, E], f16, kind="ExternalInput").ap()
    onescols_ap = nc.dram_tensor("onescols", [128, NKT], f16, kind="ExternalInput").ap()
    identh_ap = nc.dram_tensor("identh", [128, 128], f16, kind="ExternalInput").ap()
    outT_ap = nc.dram_tensor("outT", [E, L], f16, kind="ExternalOutput").ap()

    with tile.TileContext(nc) as tc:
        from contextlib import ExitStack
        with ExitStack() as ctx:
            pers = ctx.enter_context(tc.tile_pool(name="pers", bufs=1))
            work = ctx.enter_context(tc.tile_pool(name="work", bufs=1))
            biasp = ctx.enter_context(tc.tile_pool(name="bias", bufs=4))
            pp = ctx.enter_context(tc.tile_pool(name="pp", bufs=6))
            etp = ctx.enter_context(tc.tile_pool(name="etp", bufs=3))
            nrm = ctx.enter_context(tc.tile_pool(name="nrm", bufs=1))
            dramp = ctx.enter_context(tc.tile_pool(name="dram", bufs=4, space="DRAM"))
            outp = ctx.enter_context(tc.tile_pool(name="outp", bufs=3))
            # one PSUM layout for the whole kernel: no pool-transition barrier
            sp = ctx.enter_context(tc.tile_pool(name="s", bufs=2, space="PSUM"))
            yp = ctx.enter_context(tc.tile_pool(name="y", bufs=1, space="PSUM"))

            # --- proj-critical DMAs first (dispatch order matters) ---
            # x and w_proj arrive in per-chunk contiguous pieces so the proj
            # matmuls can start as soon as the first chunks land.
            xT_sb = [pers.tile([128, L], f32r, name=f"xT{e}", tag=f"xT{e}")
                     for e in range(NE)]
            wpT_sb = [pers.tile([128, 3 * C2], f32r, name=f"wpT{e}", tag=f"wpT{e}")
                      for e in range(NE)]
            for e in range(NE):
                nc.sync.dma_start(wpT_sb[e], wpT_ap[e * 128:(e + 1) * 128, :])
                nc.sync.dma_start(xT_sb[e][:, 0:1024],
                                  xT_ap[e * 128:(e + 1) * 128, 0:1024])
            for e in range(NE):
                nc.sync.dma_start(xT_sb[e][:, 1024:2048],
                                  xT_ap[e * 128:(e + 1) * 128, 1024:2048])
            wgT_sb = [pers.tile([128, C2], f32r, name=f"wgT{e}", tag=f"wgT{e}")
                      for e in range(NE)]
            for e in range(NE):
                nc.sync.dma_start(wgT_sb[e], wgT_ap[e * 128:(e + 1) * 128, :])
            bgv_sb = pers.tile([C2, 1], f32, tag="bgv")
            nc.sync.dma_start(bgv_sb, bgv_ap)
            woT_sb = pers.tile([C2, E], f16, tag="woT")
            nc.sync.dma_start(woT_sb, woT_ap)
            identh_sb = pers.tile([128, 128], f16, tag="identh")
            nc.sync.dma_start(identh_sb, identh_ap)
            # v tiles: [128 l, 130] per k-chunk: [v_h0 | ones | v_h1 | ones]
            v_all = pers.tile([128, NKT, 130], f16, tag="v_all")
            nc.sync.dma_start(v_all[:, :, 64:65], onescols_ap.unsqueeze(2))
            nc.sync.dma_start(v_all[:, :, 129:130], onescols_ap.unsqueeze(2))

            q01 = pers.tile([128, L], f16, tag="q01")
            k01 = pers.tile([128, L], f16, tag="k01")
            g_sb = pers.tile([128, L], f16, tag="g")
            ygT = pers.tile([128, L], f16, tag="ygT")
            nbias = pers.tile([128, 1], f32, tag="nbias")
            nc.vector.memset(nbias, DEVSHIFT)


            # ---------------- proj ----------------
            # e is the weight-change axis; the two inner 512-slices reuse the
            # loaded weight chunk (consecutive same-weight matmuls pipeline).
            vT01 = work.tile([128, L], f16, tag="vT01")
            dests = [q01, k01, vT01]
            for lh in range(2):
                for f in range(3):
                    ps = sp.tile([128, 1024], f32, name=f"pj{f}_{lh}", tag="s")
                    for e in range(NE):
                        for ltq in range(2):
                            nc.tensor.matmul(
                                ps[:, ltq * 512:(ltq + 1) * 512],
                                wpT_sb[e][:, f * 128:(f + 1) * 128],
                                xT_sb[e][:, lh * 1024 + ltq * 512:
                                          lh * 1024 + (ltq + 1) * 512],
                                start=(e == 0), stop=(e == NE - 1))
                    nc.vector.tensor_copy(
                        dests[f][:, lh * 1024:(lh + 1) * 1024], ps)

            # gate: g = sigmoid(wgT.T @ xT + bg) -- before the transposes so
            # the PE stream stays dense while the vT01 copy lands.
            for lh in range(2):
                ps = sp.tile([C2, 1024], f32, name=f"pg{lh}", tag="s")
                for e in range(NE):
                    for ltq in range(2):
                        nc.tensor.matmul(
                            ps[:, ltq * 512:(ltq + 1) * 512], wgT_sb[e],
                            xT_sb[e][:, lh * 1024 + ltq * 512:
                                      lh * 1024 + (ltq + 1) * 512],
                            start=(e == 0), stop=(e == NE - 1))
                nc.scalar.activation(
                    g_sb[:, lh * 1024:(lh + 1) * 1024], ps,
                    mybir.ActivationFunctionType.Sigmoid,
                    bias=bgv_sb, scale=1.0)

            # transpose vT01 -> v_all[:, kt, :]; 4 fp16 transposes share one
            # PSUM tile so the PE never ping-pongs with the copy engine.
            for g4 in range(NKT // 4):
                ps = sp.tile([128, 4, 128], f16, name=f"tr{g4}", tag="s")
                for i in range(4):
                    kt = g4 * 4 + i
                    nc.tensor.transpose(
                        ps[:, i, :], vT01[:, kt * 128:(kt + 1) * 128], identh_sb)
                nc.vector.tensor_copy(
                    v_all[:, g4 * 4:(g4 + 1) * 4, 0:64], ps[:, :, 0:64])
                nc.vector.tensor_copy(
                    v_all[:, g4 * 4:(g4 + 1) * 4, 65:129], ps[:, :, 64:128])

            # ---------------- attention: 4 passes over (q-half, head) ----------------
            # y psum double-buffered across passes so pass p+1 accumulates
            # while pass p drains through its normalization chain. The
            # q-half tail (gate mul + o_proj) is emitted one pass late so the
            # PE stream never blocks on the normalization DMA round-trip.
            # pv matmuls run LOOK k-steps behind qk so the PE never waits on
            # the add->exp chain (PE idle gaps re-throttle HAM to 1.2 GHz).
            # Each pass's normalization chains and the previous q-half's
            # tail are emitted a few steps into the NEXT pass (y psum is
            # double-buffered across passes); the injections are spread so
            # no single vector-queue bubble exceeds the pv lookahead slack.
            LOOK = 4

            def norm_chains(qhalf, h, y_ps):
                # normalization chains (softmax denominators in row 64)
                for qq in range(2):
                    qt = qhalf * 2 + qq
                    qsl = slice(qt * 512, (qt + 1) * 512)
                    sums_sb = nrm.tile([1, 512], f32,
                                       name=f"sums{qhalf}_{h}_{qq}", tag="sums")
                    nc.vector.tensor_copy(sums_sb, y_ps[qq][64:65, :])
                    dscr = dramp.tile([1, 512], f32,
                                      name=f"dscr{qhalf}_{h}_{qq}", tag="dscr")
                    nc.sync.dma_start(dscr, sums_sb)
                    sums_b = nrm.tile([64, 512], f32,
                                      name=f"sums_b{qhalf}_{h}_{qq}", tag="sums_b")
                    nc.sync.dma_start(sums_b, dscr.partition_broadcast(64))
                    rb_sb = nrm.tile([64, 512], f32, name=f"rb{qhalf}_{h}_{qq}", tag="rb")
                    nc.vector.reciprocal_approx_fast(rb_sb, sums_b)
                    if h == 0:
                        nc.vector.tensor_mul(
                            ygT[0:64, qsl], y_ps[qq][0:64, :], rb_sb)
                    else:
                        yg1 = nrm.tile([64, 512], f16,
                                       name=f"yg1_{qhalf}_{qq}", tag="yg1")
                        nc.vector.tensor_mul(yg1, y_ps[qq][0:64, :], rb_sb)
                        nc.sync.dma_start(ygT[64:128, qsl], yg1)

            def attention_pass(qhalf, h, pending=()):
                hb = h * 64
                y_ps = [yp.tile([65, 512], f32, name=f"y{qhalf}_{h}_{i}",
                                tag=f"y{i}", bufs=2) for i in range(2)]
                pend = dict(pending)
                pqueue = []
                for kt in range(NKT + LOOK):
                    if kt < NKT:
                        bias_t = biasp.tile([128, 1024], f16,
                                            name=f"bias{qhalf}_{h}_{kt}", tag="bias")
                        dma_eng = nc.gpsimd if kt % 2 == 0 else nc.sync
                        dma_eng.dma_start(
                            bias_t, biasT_ap[h, kt * 128:(kt + 1) * 128,
                                             qhalf * 1024:(qhalf + 1) * 1024])
                        s_ps = sp.tile([128, 1024], f32,
                                       name=f"s{qhalf}_{h}_{kt}", tag="s")
                        for qq in range(2):
                            qs = qhalf * 1024 + qq * 512
                            nc.tensor.matmul(
                                s_ps[:, qq * 512:(qq + 1) * 512],
                                k01[hb:hb + 64, kt * 128:(kt + 1) * 128],
                                q01[hb:hb + 64, qs:qs + 512],
                                start=True, stop=True)
                        # multiplicative bias: p = exp(s) * exp(bias-7)*mask
                        # (host-precomputed fp16). The fp16 multiply runs at
                        # 2x DVE rate vs the old f32 PSUM add, the exp no
                        # longer waits on the bias tile, and masking is an
                        # exact zero. The e^-7 shift keeps p in fp16 range;
                        # denominators scale identically so ratios are exact.
                        e_t = etp.tile([128, 1024], f16,
                                       name=f"e{qhalf}_{h}_{kt}", tag="e")
                        nc.scalar.activation(
                            e_t, s_ps, mybir.ActivationFunctionType.Exp,
                            bias=nbias)
                        p_t = pp.tile([128, 1024], f16,
                                      name=f"p{qhalf}_{h}_{kt}", tag="p")
                        nc.vector.tensor_mul(p_t, e_t, bias_t)
                        pqueue.append((kt, p_t))
                    if kt >= LOOK:
                        pkt, p_t = pqueue[kt - LOOK]
                        for qq in range(2):
                            nc.tensor.matmul(
                                y_ps[qq],
                                v_all[:, pkt, h * 65:(h + 1) * 65],
                                p_t[:, qq * 512:(qq + 1) * 512],
                                start=(pkt == 0), stop=(pkt == NKT - 1))
                    if kt in pend:
                        pend.pop(kt)()
                return lambda: norm_chains(qhalf, h, y_ps)

            def qhalf_tail(qhalf, eo_range, gate=False):
                # gate multiply + o_proj partial for this q-half
                if gate:
                    for qq in range(2):
                        qt = qhalf * 2 + qq
                        qsl = slice(qt * 512, (qt + 1) * 512)
                        nc.vector.tensor_mul(ygT[:, qsl], ygT[:, qsl], g_sb[:, qsl])
                for eo in eo_range:
                    ps = sp.tile([128, 1024], f32, name=f"po{qhalf}_{eo}", tag="s")
                    for qq in range(2):
                        qt = qhalf * 2 + qq
                        nc.tensor.matmul(
                            ps[:, qq * 512:(qq + 1) * 512],
                            woT_sb[:, eo * 128:(eo + 1) * 128],
                            ygT[:, qt * 512:(qt + 1) * 512],
                            start=True, stop=True)
                    ot = outp.tile([128, 1024], f16, name=f"ot{qhalf}_{eo}", tag="ot")
                    if eo % 2 == 0:
                        nc.vector.tensor_copy(ot, ps)
                    else:
                        nc.scalar.copy(ot, ps)
                    nc.sync.dma_start(
                        outT_ap[eo * 128:(eo + 1) * 128,
                                qhalf * 1024:(qhalf + 1) * 1024], ot)

            # head 1 first within each q-half: the final pass (head 0) has
            # the shift-free normalization chain, shortening the tail.
            # Pass P's norm chains are injected early into pass P+1, and the
            # first q-half's tail is split across two injection points so no
            # vector-queue bubble outruns the pv lookahead slack.
            c01 = attention_pass(0, 1)
            c00 = attention_pass(0, 0, pending={1: c01})
            c11 = attention_pass(1, 1, pending={1: c00})
            c10 = attention_pass(1, 0, pending={
                1: c11,
                5: lambda: qhalf_tail(0, range(0, 4), gate=True),
                9: lambda: qhalf_tail(0, range(4, 8)),
            })
            c10()
            qhalf_tail(1, range(NE), gate=True)

    nc.compile()
    return nc


def kernel(x, mask, bias, w_proj, w_o, b_o, w_g, b_g):
    x = np.asarray(x, dtype=np.float32)
    mask = np.asarray(mask)
    bias = np.asarray(bias, dtype=np.float32)
    w_proj = np.asarray(w_proj, dtype=np.float32)
    w_o = np.asarray(w_o, dtype=np.float32)
    b_o = np.asarray(b_o, dtype=np.float32)
    w_g = np.asarray(w_g, dtype=np.float32)
    b_g = np.asarray(b_g, dtype=np.float32)

    if _compiled[0] is None:
        _compiled[0] = _build()
    nc = _compiled[0]

    xT = np.ascontiguousarray(x[0].T)                      # [E, L]
    onescols = np.ones((128, NKT), dtype=np.float16)
    identh = np.eye(128, dtype=np.float16)

    in_maps = []
    for c in range(N_CORES):
        heads = [c * HPC + i for i in range(HPC)]
        wpT = np.empty((E, 3 * C2), dtype=np.float32)
        for i, h in enumerate(heads):
            r0 = h * 3 * HW
            wpT[:, 0 * C2 + i * HW: 0 * C2 + (i + 1) * HW] = \
                w_proj[r0: r0 + HW].T * SCALE               # q, pre-scaled
            wpT[:, 1 * C2 + i * HW: 1 * C2 + (i + 1) * HW] = \
                w_proj[r0 + HW: r0 + 2 * HW].T              # k
            wpT[:, 2 * C2 + i * HW: 2 * C2 + (i + 1) * HW] = \
                w_proj[r0 + 2 * HW: r0 + 3 * HW].T          # v
        biasT = np.ascontiguousarray(
            bias[0, :, :, heads].transpose(0, 2, 1))        # [2, Lk, Lq]
        # multiplicative form: exp(bias + HOSTSHIFT), masked keys exactly 0
        biasT = np.exp(biasT + HOSTSHIFT)
        biasT *= mask[0].astype(np.float32)[None, :, None]
        biasT = biasT.astype(np.float16)
        cols = slice(c * C2, (c + 1) * C2)
        wgT = np.ascontiguousarray(w_g[cols, :].T)          # [E, C2]
        bgv = np.ascontiguousarray(b_g[cols, None])         # [C2, 1]
        woT = np.ascontiguousarray(w_o[:, cols].T).astype(np.float16)  # [C2, E]
        in_maps.append({
            "xT": xT, "wpT": wpT, "biasT": biasT, "wgT": wgT,
            "bgv": bgv, "woT": woT, "onescols": onescols, "identh": identh,
        })

    res = run_bass_kernel_spmd(nc, in_maps, list(range(N_CORES)))
    acc = res.results[0]["outT"].astype(np.float64)
    for c in range(1, N_CORES):
        acc += res.results[c]["outT"]
    out = acc.T.astype(np.float32) + b_o[None, :]
    return out[None]  # [B, L, E]


# revision 50
# speedup vs baseline: 1.4497x; 1.0669x over previous
"""Trainium2 Bass kernel for nn_Attention_79645873537262.

Dense attention with per-head bias, key masking, sigmoid gate:
  t = x @ w_proj.T; per head: q,k,v
  a = softmax(scale*q@k.T + bias + mask); y = a@v
  y = sigmoid(x@w_g.T + b_g) * y;  out = y @ w_o.T + b_o

Sharding: tensor-parallel over heads, 2 heads per core on 8 cores.
Each core runs a fully independent program (no collectives): it computes
its 2 heads' attention plus its 128-column slice of the gate, and a
partial o_proj (contribution of its 128 y-columns to all 1024 outputs).
The host sums the 8 partial outputs and adds b_o (the "all-reduce").

On-device layout is transposed ("scores.T" flash style):
  scores.T[k,q] = kT.T@qT in PSUM; bias (pre-masked, pre-transposed,
  fp16) is added IN PLACE by the vector engine (PSUM += SBUF bias) --
  this keeps the PE out of the bias path entirely (the old ident@bias
  seeding cost 128 extra matmuls ~60us of PE time); exp on ScalarE
  (no max-subtraction: logits are ~N(0,2), |logit| < ~14 so exp is
  safe); y.T ext = [v | ones].T @ p gives y.T rows 0..63 and the
  softmax denominator in row 64. Normalization multiplies by a
  broadcast reciprocal (DMA DRAM round-trip to cross partitions).
All matmuls run in float32r.

Perf notes (from NTFF traces): PE streams 512-col matmuls at ~1.1
cols/ns (p-state MID) and HAM throttling halves that in windows, so
the only real lever is fewer matmul instructions and no PE idle.
Bias stream alternates Sync/GpSimd DGE queues; v transposes batch 4
per PSUM tile with 2 wide copies; per-q-half tails run one pass late;
within each q-half head 1 runs first so the final pass's norm chain
(head 0) skips the SBUF partition-shift DMA. Partial outputs are fp16
(halves the tail output-DMA drain; host accumulates in f64).
"""
import sys
import numpy as np
import ml_dtypes

try:
    import concourse.bass as bass
except ImportError:
    sys.path.insert(0, "/opt/trn_rl_repo")
    import concourse.bass as bass

import concourse.tile as tile
from concourse import bacc, mybir
from concourse.bass_utils import run_bass_kernel_spmd

B, L, E, H = 1, 2048, 1024, 16
HW = E // H                # 64
SCALE = HW ** -0.5
N_CORES = 8
HPC = H // N_CORES         # 2 heads per core
C2 = HPC * HW              # 128 y-columns per core
MASK_NEG = -60.0           # exp(-60 + max_bias) ~ 1e-23: dead keys vanish
# p = exp(s + DEVSHIFT) * exp(bias + HOSTSHIFT): the total e^-7 shift keeps
# max p = exp(15.06 - 7) in fp16; the device part keeps exp(s_max=13.43 - 4)
# in fp16 too. Denominators scale identically, so softmax ratios are exact.
DEVSHIFT = -4.0
HOSTSHIFT = -3.0

f32 = mybir.dt.float32
f32r = mybir.dt.float32r
f16 = mybir.dt.float16
bf16 = mybir.dt.bfloat16

NE = E // 128              # 8 contraction chunks
NQ = L // 512              # 4 q-tiles of 512
NKT = L // 128             # 16 k-chunks of 128

_compiled = [None]
DEBUG = False


def _build():
    nc = bacc.Bacc("TRN2", target_bir_lowering=False, debug=False,
                   num_devices=N_CORES)

    xT_ap = nc.dram_tensor("xT", [E, L], bf16, kind="ExternalInput").ap()
    wpT_ap = nc.dram_tensor("wpT", [E, 3 * C2], bf16, kind="ExternalInput").ap()
    biasT_ap = nc.dram_tensor("biasT", [HPC, L, L], f16, kind="ExternalInput").ap()
    wgT_ap = nc.dram_tensor("wgT", [E, C2], bf16, kind="ExternalInput").ap()
    bgv_ap = nc.dram_tensor("bgv", [C2, 1], f32, kind="ExternalInput").ap()
    woT_ap = nc.dram_tensor("woT", [C2, E], f16, kind="ExternalInput").ap()
    onescols_ap = nc.dram_tensor("onescols", [128, NKT], f16, kind="ExternalInput").ap()
    identh_ap = nc.dram_tensor("identh", [128, 128], f16, kind="ExternalInput").ap()
    outT_ap = nc.dram_tensor("outT", [E, L], f16, kind="ExternalOutput").ap()

    with tile.TileContext(nc) as tc:
        from contextlib import ExitStack
        with ExitStack() as ctx:
            pers = ctx.enter_context(tc.tile_pool(name="pers", bufs=1))
            work = ctx.enter_context(tc.tile_pool(name="work", bufs=1))
            biasp = ctx.enter_context(tc.tile_pool(name="bias", bufs=4))
            pp = ctx.enter_context(tc.tile_pool(name="pp", bufs=6))
            etp = ctx.enter_context(tc.tile_pool(name="etp", bufs=3))
            nrm = ctx.enter_context(tc.tile_pool(name="nrm", bufs=1))
            dramp = ctx.enter_context(tc.tile_pool(name="dram", bufs=4, space="DRAM"))
            outp = ctx.enter_context(tc.tile_pool(name="outp", bufs=3))
            # one PSUM layout for the whole kernel: no pool-transition barrier
            sp = ctx.enter_context(tc.tile_pool(name="s", bufs=2, space="PSUM"))
            yp = ctx.enter_context(tc.tile_pool(name="y", bufs=1, space="PSUM"))

            # --- proj-critical DMAs first (dispatch order matters) ---
            # x and w_proj arrive in per-chunk contiguous pieces so the proj
            # matmuls can start as soon as the first chunks land; the
            # critical dispatches fan out across three hardware DGE queues.
            xT_sb = [pers.tile([128, L], bf16, name=f"xT{e}", tag=f"xT{e}")
                     for e in range(NE)]
            wpT_sb = [pers.tile([128, 3 * C2], bf16, name=f"wpT{e}", tag=f"wpT{e}")
                      for e in range(NE)]
            for e in range(NE):
                nc.sync.dma_start(wpT_sb[e], wpT_ap[e * 128:(e + 1) * 128, :])
                dma_eng = nc.gpsimd if e % 2 == 0 else nc.scalar
                dma_eng.dma_start(xT_sb[e][:, 0:1024],
                                  xT_ap[e * 128:(e + 1) * 128, 0:1024])
            for e in range(NE):
                nc.sync.dma_start(xT_sb[e][:, 1024:2048],
                                  xT_ap[e * 128:(e + 1) * 128, 1024:2048])
            wgT_sb = [pers.tile([128, C2], bf16, name=f"wgT{e}", tag=f"wgT{e}")
                      for e in range(NE)]
            for e in range(NE):
                nc.scalar.dma_start(wgT_sb[e], wgT_ap[e * 128:(e + 1) * 128, :])
            bgv_sb = pers.tile([C2, 1], f32, tag="bgv")
            nc.sync.dma_start(bgv_sb, bgv_ap)
            woT_sb = pers.tile([C2, E], f16, tag="woT")
            nc.sync.dma_start(woT_sb, woT_ap)
            identh_sb = pers.tile([128, 128], f16, tag="identh")
            nc.sync.dma_start(identh_sb, identh_ap)
            # v tiles: [128 l, 130] per k-chunk: [v_h0 | ones | v_h1 | ones]
            v_all = pers.tile([128, NKT, 130], f16, tag="v_all")
            nc.sync.dma_start(v_all[:, :, 64:65], onescols_ap.unsqueeze(2))
            nc.sync.dma_start(v_all[:, :, 129:130], onescols_ap.unsqueeze(2))

            q01 = pers.tile([128, L], f16, tag="q01")
            k01 = pers.tile([128, L], f16, tag="k01")
            g_sb = pers.tile([128, L], f16, tag="g")
            ygT = pers.tile([128, L], f16, tag="ygT")
            nbias = pers.tile([128, 1], f32, tag="nbias")
            nc.vector.memset(nbias, DEVSHIFT)


            # ---------------- proj ----------------
            # e is the weight-change axis; the two inner 512-slices reuse the
            # loaded weight chunk (consecutive same-weight matmuls pipeline).
            vT01 = work.tile([128, L], f16, tag="vT01")
            dests = [q01, k01, vT01]
            for lh in range(2):
                for f in range(3):
                    ps = sp.tile([128, 1024], f32, name=f"pj{f}_{lh}", tag="s")
                    for e in range(NE):
                        for ltq in range(2):
                            nc.tensor.matmul(
                                ps[:, ltq * 512:(ltq + 1) * 512],
                                wpT_sb[e][:, f * 128:(f + 1) * 128],
                                xT_sb[e][:, lh * 1024 + ltq * 512:
                                          lh * 1024 + (ltq + 1) * 512],
                                start=(e == 0), stop=(e == NE - 1))
                    nc.vector.tensor_copy(
                        dests[f][:, lh * 1024:(lh + 1) * 1024], ps)

            # gate: g = sigmoid(wgT.T @ xT + bg) -- before the transposes so
            # the PE stream stays dense while the vT01 copy lands.
            for lh in range(2):
                ps = sp.tile([C2, 1024], f32, name=f"pg{lh}", tag="s")
                for e in range(NE):
                    for ltq in range(2):
                        nc.tensor.matmul(
                            ps[:, ltq * 512:(ltq + 1) * 512], wgT_sb[e],
                            xT_sb[e][:, lh * 1024 + ltq * 512:
                                      lh * 1024 + (ltq + 1) * 512],
                            start=(e == 0), stop=(e == NE - 1))
                nc.scalar.activation(
                    g_sb[:, lh * 1024:(lh + 1) * 1024], ps,
                    mybir.ActivationFunctionType.Sigmoid,
                    bias=bgv_sb, scale=1.0)

            # transpose vT01 -> v_all[:, kt, :]; 4 fp16 transposes share one
            # PSUM tile so the PE never ping-pongs with the copy engine.
            for g4 in range(NKT // 4):
                ps = sp.tile([128, 4, 128], f16, name=f"tr{g4}", tag="s")
                for i in range(4):
                    kt = g4 * 4 + i
                    nc.tensor.transpose(
                        ps[:, i, :], vT01[:, kt * 128:(kt + 1) * 128], identh_sb)
                nc.vector.tensor_copy(
                    v_all[:, g4 * 4:(g4 + 1) * 4, 0:64], ps[:, :, 0:64])
                nc.vector.tensor_copy(
                    v_all[:, g4 * 4:(g4 + 1) * 4, 65:129], ps[:, :, 64:128])

            # ---------------- attention: 4 passes over (q-half, head) ----------------
            # y psum double-buffered across passes so pass p+1 accumulates
            # while pass p drains through its normalization chain. The
            # q-half tail (gate mul + o_proj) is emitted one pass late so the
            # PE stream never blocks on the normalization DMA round-trip.
            # pv matmuls run LOOK k-steps behind qk so the PE never waits on
            # the add->exp chain (PE idle gaps re-throttle HAM to 1.2 GHz).
            # Each pass's normalization chains and the previous q-half's
            # tail are emitted a few steps into the NEXT pass (y psum is
            # double-buffered across passes); the injections are spread so
            # no single vector-queue bubble exceeds the pv lookahead slack.
            LOOK = 4

            def norm_chains(qhalf, h, y_ps):
                # normalization chains (softmax denominators in row 64)
                for qq in range(2):
                    qt = qhalf * 2 + qq
                    qsl = slice(qt * 512, (qt + 1) * 512)
                    sums_sb = nrm.tile([1, 512], f32,
                                       name=f"sums{qhalf}_{h}_{qq}", tag="sums")
                    nc.vector.tensor_copy(sums_sb, y_ps[qq][64:65, :])
                    dscr = dramp.tile([1, 512], f32,
                                      name=f"dscr{qhalf}_{h}_{qq}", tag="dscr")
                    nc.sync.dma_start(dscr, sums_sb)
                    sums_b = nrm.tile([64, 512], f32,
                                      name=f"sums_b{qhalf}_{h}_{qq}", tag="sums_b")
                    nc.sync.dma_start(sums_b, dscr.partition_broadcast(64))
                    rb_sb = nrm.tile([64, 512], f32, name=f"rb{qhalf}_{h}_{qq}", tag="rb")
                    nc.vector.reciprocal_approx_fast(rb_sb, sums_b)
                    if h == 0:
                        nc.vector.tensor_mul(
                            ygT[0:64, qsl], y_ps[qq][0:64, :], rb_sb)
                    else:
                        yg1 = nrm.tile([64, 512], f16,
                                       name=f"yg1_{qhalf}_{qq}", tag="yg1")
                        nc.vector.tensor_mul(yg1, y_ps[qq][0:64, :], rb_sb)
                        nc.sync.dma_start(ygT[64:128, qsl], yg1)

            def attention_pass(qhalf, h, pending=()):
                hb = h * 64
                y_ps = [yp.tile([65, 512], f32, name=f"y{qhalf}_{h}_{i}",
                                tag=f"y{i}", bufs=2) for i in range(2)]
                pend = dict(pending)
                pqueue = []
                for kt in range(NKT + LOOK):
                    if kt < NKT:
                        bias_t = biasp.tile([128, 1024], f16,
                                            name=f"bias{qhalf}_{h}_{kt}", tag="bias")
                        dma_eng = nc.gpsimd if kt % 2 == 0 else nc.sync
                        dma_eng.dma_start(
                            bias_t, biasT_ap[h, kt * 128:(kt + 1) * 128,
                                             qhalf * 1024:(qhalf + 1) * 1024])
                        s_ps = sp.tile([128, 1024], f32,
                                       name=f"s{qhalf}_{h}_{kt}", tag="s")
                        for qq in range(2):
                            qs = qhalf * 1024 + qq * 512
                            nc.tensor.matmul(
                                s_ps[:, qq * 512:(qq + 1) * 512],
                                k01[hb:hb + 64, kt * 128:(kt + 1) * 128],
                                q01[hb:hb + 64, qs:qs + 512],
                                start=True, stop=True)
                        # multiplicative bias: p = exp(s) * exp(bias-7)*mask
                        # (host-precomputed fp16). The fp16 multiply runs at
                        # 2x DVE rate vs the old f32 PSUM add, the exp no
                        # longer waits on the bias tile, and masking is an
                        # exact zero. The e^-7 shift keeps p in fp16 range;
                        # denominators scale identically so ratios are exact.
                        e_t = etp.tile([128, 1024], f16,
                                       name=f"e{qhalf}_{h}_{kt}", tag="e")
                        nc.scalar.activation(
                            e_t, s_ps, mybir.ActivationFunctionType.Exp,
                            bias=nbias)
                        p_t = pp.tile([128, 1024], f16,
                                      name=f"p{qhalf}_{h}_{kt}", tag="p")
                        nc.vector.tensor_mul(p_t, e_t, bias_t)
                        pqueue.append((kt, p_t))
                    if kt >= LOOK:
                        pkt, p_t = pqueue[kt - LOOK]
                        for qq in range(2):
                            nc.tensor.matmul(
                                y_ps[qq],
                                v_all[:, pkt, h * 65:(h + 1) * 65],
                                p_t[:, qq * 512:(qq + 1) * 512],
                                start=(pkt == 0), stop=(pkt == NKT - 1))
                    if kt in pend:
                        pend.pop(kt)()
                return lambda: norm_chains(qhalf, h, y_ps)

            def qhalf_tail(qhalf, eo_range, gate=False):
                # gate multiply + o_proj partial for this q-half
                if gate:
                    for qq in range(2):
                        qt = qhalf * 2 + qq
                        qsl = slice(qt * 512, (qt + 1) * 512)
                        nc.vector.tensor_mul(ygT[:, qsl], ygT[:, qsl], g_sb[:, qsl])
                for eo in eo_range:
                    ps = sp.tile([128, 1024], f32, name=f"po{qhalf}_{eo}", tag="s")
                    for qq in range(2):
                        qt = qhalf * 2 + qq
                        nc.tensor.matmul(
                            ps[:, qq * 512:(qq + 1) * 512],
                            woT_sb[:, eo * 128:(eo + 1) * 128],
                            ygT[:, qt * 512:(qt + 1) * 512],
                            start=True, stop=True)
                    ot = outp.tile([128, 1024], f16, name=f"ot{qhalf}_{eo}", tag="ot")
                    nc.vector.tensor_copy(ot, ps)
                    nc.sync.dma_start(
                        outT_ap[eo * 128:(eo + 1) * 128,
                                qhalf * 1024:(qhalf + 1) * 1024], ot)

            # head 1 first within each q-half: the final pass (head 0) has
            # the shift-free normalization chain, shortening the tail.
            # Pass P's norm chains are injected early into pass P+1, and the
            # first q-half's tail is split across two injection points so no
            # vector-queue bubble outruns the pv lookahead slack.
            c01 = attention_pass(0, 1)
            c00 = attention_pass(0, 0, pending={1: c01})
            c11 = attention_pass(1, 1, pending={1: c00})
            c10 = attention_pass(1, 0, pending={
                1: c11,
                5: lambda: qhalf_tail(0, range(0, 4), gate=True),
                9: lambda: qhalf_tail(0, range(4, 8)),
            })
            c10()
            qhalf_tail(1, range(NE), gate=True)

    nc.compile()
    return nc


def kernel(x, mask, bias, w_proj, w_o, b_o, w_g, b_g):
    x = np.asarray(x, dtype=np.float32)
    mask = np.asarray(mask)
    bias = np.asarray(bias, dtype=np.float32)
    w_proj = np.asarray(w_proj, dtype=np.float32)
    w_o = np.asarray(w_o, dtype=np.float32)
    b_o = np.asarray(b_o, dtype=np.float32)
    w_g = np.asarray(w_g, dtype=np.float32)
    b_g = np.asarray(b_g, dtype=np.float32)

    if _compiled[0] is None:
        _compiled[0] = _build()
    nc = _compiled[0]

    xT = np.ascontiguousarray(x[0].T).astype(ml_dtypes.bfloat16)  # [E, L]
    onescols = np.ones((128, NKT), dtype=np.float16)
    identh = np.eye(128, dtype=np.float16)

    in_maps = []
    for c in range(N_CORES):
        heads = [c * HPC + i for i in range(HPC)]
        wpT = np.empty((E, 3 * C2), dtype=np.float32)
        for i, h in enumerate(heads):
            r0 = h * 3 * HW
            wpT[:, 0 * C2 + i * HW: 0 * C2 + (i + 1) * HW] = \
                w_proj[r0: r0 + HW].T * SCALE               # q, pre-scaled
            wpT[:, 1 * C2 + i * HW: 1 * C2 + (i + 1) * HW] = \
                w_proj[r0 + HW: r0 + 2 * HW].T              # k
            wpT[:, 2 * C2 + i * HW: 2 * C2 + (i + 1) * HW] = \
                w_proj[r0 + 2 * HW: r0 + 3 * HW].T          # v
        biasT = np.ascontiguousarray(
            bias[0, :, :, heads].transpose(0, 2, 1))        # [2, Lk, Lq]
        # multiplicative form: exp(bias + HOSTSHIFT), masked keys exactly 0
        biasT = np.exp(biasT + HOSTSHIFT)
        biasT *= mask[0].astype(np.float32)[None, :, None]
        biasT = biasT.astype(np.float16)
        cols = slice(c * C2, (c + 1) * C2)
        wgT = np.ascontiguousarray(w_g[cols, :].T).astype(ml_dtypes.bfloat16)
        bgv = np.ascontiguousarray(b_g[cols, None])         # [C2, 1]
        woT = np.ascontiguousarray(w_o[:, cols].T).astype(np.float16)  # [C2, E]
        in_maps.append({
            "xT": xT, "wpT": wpT.astype(ml_dtypes.bfloat16), "biasT": biasT,
            "wgT": wgT,
            "bgv": bgv, "woT": woT, "onescols": onescols, "identh": identh,
        })

    res = run_bass_kernel_spmd(nc, in_maps, list(range(N_CORES)))
    acc = res.results[0]["outT"].astype(np.float64)
    for c in range(1, N_CORES):
        acc += res.results[c]["outT"]
    out = acc.T.astype(np.float32) + b_o[None, :]
    return out[None]  # [B, L, E]
